# revision 42
# baseline (speedup 1.0000x reference)
"""Causal multi-head attention (RoPE) on 8 TRN2 NeuronCores.

Problem: x[2,2048,2048] -> qkv proj -> rope -> causal attention (16 heads,
head_dim 128) -> output proj + bias. Sharding: (batch, head-group) across the
8 cores - core c handles batch c//4 and heads 4*(c%4)..4*(c%4)+3. Each core
computes a partial output projection over its heads' channels; the host sums
the 4 partials per batch and adds b_o.

Mixed-precision pipeline (tolerance 2e-2; this lands ~3.8e-3):
  - QKV projection and output projection run in fp8 (e4m3) with a hi/lo
    3-term split (W1X1 + W1X2 + W2X1, weights pre-scaled by 64 into the
    e4m3 normal range) using DoubleRow matmuls: each instruction contracts
    2x128 rows at 0.5 cycles per output column - 1.33x the f32r rate at
    bf16-class accuracy. The 64x scale is unwound via the exp() scale
    (weights enter scores twice) and a 1/64 factor folded into the
    softmax-sum ones vector and the output eviction.
  - Attention (scores, exp, AV) runs in bf16: 1 cyc/col with no 256-col
    floor, so causal narrowing works at 128-col granularity; q/k/v evict
    from PSUM to bf16 SBUF and stay resident.
  - Softmax: scores stay transposed s^T[tk,tq]; row sums l accumulate via
    a [128,1] ones-column matmul per tile; 1/l (DVE reciprocal, hoisted to
    the head boundary) broadcasts across partitions with a gpsimd
    partition_broadcast instead of a PE matmul.

Schedule (the perf comes from overlap, all verified against TimelineSim):
  - QKV runs in 2 passes over the contraction (pair-chunks 0-1, then 2-7)
    so PE starts ~4us in, DMA-paced; pass-B partials merge into SBUF via
    DVE adds. All weight/x DMAs are issued in compute order up front.
  - Whole-row rope (2 big half-swap DMAs + 3 elementwise ops per q/k row)
    runs during QKV pass B via a per-group callback, so attention starts
    immediately after the projection.
  - The output projection for block jb-1 and 4 deferred v-tiles are
    interleaved into block jb's attention inner loops (one thunk every
    few iterations) to fill the PE bubbles left by the score->exp->mask->AV
    dependency chain; scores run 4 iterations ahead of AV, and each head's
    ctx normalize/fp8-split chain is deferred into the next head's window.
  - Output rows accumulate per 128-token tile in SBUF and ship as single
    whole-row DMAs (except the last row, which ships per-block to shorten
    the teardown tail).
"""
import math

import numpy as np
import ml_dtypes

import concourse.bacc as bacc
import concourse.mybir as mybir
import concourse.tile as tile
from concourse.bass_utils import run_bass_kernel_spmd

P = 128           # partitions / head_dim
T = 2048          # context length
C = 2048          # d_model
NTT = T // P      # 16 token tiles
NB = T // 512     # 4 query blocks of 512
HPC = 4           # heads per core
NPAIR = C // 256  # 8 DoubleRow contraction pair-chunks
NCORES = 8
WS = 64.0         # fp8 weight pre-scale
SCALE = 1.0 / math.sqrt(P)
ESC = SCALE / (WS * WS)   # exp() scale: scores carry WS^2

F32 = mybir.dt.float32
BF = mybir.dt.bfloat16
F8 = mybir.dt.float8e4
EXP = mybir.ActivationFunctionType.Exp
MULT = mybir.AluOpType.mult
ADD = mybir.AluOpType.add
SUB = mybir.AluOpType.subtract
DR = mybir.MatmulPerfMode.DoubleRow

_CACHE = {}


def _phase1(nc, tc, dram, qk_sb, v_sb, const_loads, rope0, wvpool,
            xtail_loads):
    """QKV projection: fp8 hi/lo 3-term DoubleRow. Pass A covers pair-chunks
    0-1 (so PE starts ~4us in, DMA-paced), pass B covers 2-7; pass-B partials
    merge into bf16 SBUF via DVE adds. rope0(part, h) is invoked after each
    pass-B group so block-0 rope chains run during pass B."""
    x1, x2, wq1, wq2, wk1, wk2, wv1, wv2 = dram
    PASSES = (list(range(0, 2)), list(range(2, 8)))
    with (
        tc.tile_pool(name="xp", bufs=1) as xpool,
        tc.tile_pool(name="wp", bufs=1) as wpool,
        tc.tile_pool(name="psqk", bufs=4, space="PSUM") as psqk,
        tc.tile_pool(name="psv", bufs=3, space="PSUM") as psv,
    ):
        wten = {"q": (wq1, wq2), "k": (wk1, wk2)}
        groups = [(h, part) for h in range(HPC) for part in ("q", "k")]

        def load_w(gi, ps):
            h, part = groups[gi]
            js = PASSES[ps]
            jsl = slice(js[0], js[-1] + 1)
            w_sb = {}
            for lv in (1, 2):
                t_ = wpool.tile([P, len(js), 2, P], F8, tag=f"w{ps}_{gi}_{lv}",
                                bufs=1, name=f"w{part}{h}p{ps}_{lv}")
                nc.sync.dma_start(t_[:], wten[part][lv - 1][h][:, jsl])
                w_sb[lv] = t_
            return w_sb

        wq_pref = {(0, 0): load_w(0, 0)}
        xt = {}

        def load_x(js_):
            for j in js_:
                for lv, ten in ((1, x1), (2, x2)):
                    t_ = xpool.tile([P, 2, T], F8, tag=f"x{lv}_{j}",
                                    bufs=1, name=f"x{lv}_{j}")
                    nc.sync.dma_start(t_[:], ten[j])
                    xt[(lv, j)] = t_

        load_x(PASSES[0])
        for gi in range(1, len(groups)):
            wq_pref[(gi, 0)] = load_w(gi, 0)
        wv_sb = {}
        for lv, ten in ((1, wv1), (2, wv2)):
            t_ = wvpool.tile([P, NPAIR, 2, HPC * P], F8, tag=f"wv{lv}")
            nc.sync.dma_start(t_[:], ten)
            wv_sb[lv] = t_
        load_x(PASSES[1])
        const_loads()
        xtail_loads()
        for gi in range(len(groups)):
            wq_pref[(gi, 1)] = load_w(gi, 1)

        for ps in range(2):
            js = PASSES[ps]
            nmm = 3 * len(js)
            for gi, (h, part) in enumerate(groups):
                w_sb = wq_pref.pop((gi, ps))
                for nb in range(NB):
                    tsl = slice(nb * 512, (nb + 1) * 512)
                    acc = psqk.tile([P, 512], F32, tag="qk", bufs=4)
                    n = 0
                    order = ([(jj, t) for jj in range(len(js)) for t in range(3)]
                             if ps == 0 else
                             [(jj, t) for t in range(3) for jj in range(len(js))])
                    terms = ((1, 1), (1, 2), (2, 1))
                    for jj, t in order:
                        wl, xl = terms[t]
                        nc.tensor.matmul(
                            acc[:], w_sb[wl][:, jj], xt[(xl, js[jj])][:, :, tsl],
                            start=(n == 0), stop=(n == nmm - 1),
                            perf_mode=DR)
                        n += 1
                    dst = qk_sb[(part, h)]
                    if ps == 0:
                        nc.scalar.copy(dst[:, tsl], acc[:])
                    else:
                        nc.vector.tensor_tensor(dst[:, tsl], dst[:, tsl],
                                                acc[:], op=ADD)
                # two v token-tiles after each q/k group (the last four
                # pass-B tiles are deferred into block-0 attention)
                for tt in (2 * gi, 2 * gi + 1):
                    if ps == 1 and tt >= 12:
                        continue
                    ssl = slice(tt * P, (tt + 1) * P)
                    vacc = psv.tile([P, 512], F32, tag="v", bufs=3)
                    n = 0
                    for xl, wl in ((1, 1), (1, 2), (2, 1)):
                        for jj, j in enumerate(js):
                            nc.tensor.matmul(
                                vacc[:], xt[(xl, j)][:, :, ssl], wv_sb[wl][:, j],
                                start=(n == 0), stop=(n == nmm - 1),
                                perf_mode=DR)
                            n += 1
                    if ps == 0:
                        nc.scalar.copy(v_sb[tt][:], vacc[:])
                    else:
                        nc.vector.tensor_tensor(v_sb[tt][:], v_sb[tt][:],
                                                vacc[:], op=ADD)
                if ps == 1:
                    rope0(part, h)
    return wv_sb


def _attention_head(nc, pools, qk_sb, v_sb, consts, jb, h, interleave,
                    head_start=None):
    """Scores/exp/mask/AV/l for one (jb, h), with score pipelining and
    outproj interleave. head_start (the previous head's deferred
    normalize chain) is emitted after this head's first two scores so
    its PE/DVE ops hide behind fresh score work."""
    pss, psc, psl, ppool, rrpool = pools
    tri_sb, onescol_sb = consts
    qT = qk_sb[("q", h)]
    qsl = slice(jb * 512, (jb + 1) * 512)
    nt = 4 * (jb + 1)
    ctx_ps = psc.tile([P, 512], F32, tag="ctx", bufs=2)
    l_ps = psl.tile([1, 512], F32, tag="l", bufs=1)

    def score(i):
        r = i - 4 * jb
        c0 = max(0, r * P)
        osl = slice(c0, 512)
        sps = pss.tile([P, 512], F32, tag="s", bufs=2)
        kT = qk_sb[("k", h)]
        nc.tensor.matmul(sps[:, osl], kT[:, i * P:(i + 1) * P],
                         qT[:, jb * 512 + c0:(jb + 1) * 512],
                         start=True, stop=True)
        pt = ppool.tile([P, 512], BF, tag="pt", bufs=5)
        nc.scalar.activation(pt[:, osl], sps[:, osl], EXP, scale=ESC)
        if r >= 0:
            dsl = slice(r * P, (r + 1) * P)
            nc.gpsimd.tensor_tensor(pt[:, dsl], pt[:, dsl], tri_sb[:], op=MULT)
        return pt, c0

    ahead = 4
    queue = [score(i) for i in range(min(ahead, nt))]
    for i in range(nt):
        pt, c0 = queue.pop(0)
        if i + ahead < nt:
            queue.append(score(i + ahead))
        if i == 0 and head_start is not None:
            head_start()
        osl = slice(c0, 512)
        nc.tensor.matmul(ctx_ps[:, osl], v_sb[i][:, h * P:(h + 1) * P],
                         pt[:, osl], start=(i == 0), stop=(i == nt - 1))
        nc.tensor.matmul(l_ps[:, osl], onescol_sb[:], pt[:, osl],
                         start=(i == 0), stop=(i == nt - 1))
        interleave()
    rinv_row = rrpool.tile([1, 512], BF, tag="rr", bufs=2)
    with nc.allow_low_precision(reason="softmax 1/l bf16"):
        nc.vector.reciprocal(rinv_row[:], l_ps[:])
    return ctx_ps, rinv_row


def _normalize_ctx(nc, pools, consts, ctx_ps, rinv_row, c1t, c2t, sl):
    """broadcast 1/l -> normalize -> split ctx into fp8 hi/lo pair slots."""
    psb, rrpool, cxspool, cxnpool = pools
    onesrow_sb, = consts
    bps = rrpool.tile([P, 512], BF, tag="bb", bufs=2, name="bps_sb")
    nc.gpsimd.partition_broadcast(bps[:], rinv_row[:])
    cvt = cxspool.tile([P, 512], F32, tag="cvt", bufs=2)
    nc.vector.tensor_copy(cvt[:], ctx_ps[:])
    ctxn = cxnpool.tile([P, 512], F32, tag="cxn", bufs=2)
    nc.vector.tensor_tensor(ctxn[:], cvt[:], bps[:], op=MULT)
    nc.vector.tensor_copy(c1t[:, sl], ctxn[:])
    nc.vector.tensor_tensor(c2t[:, sl], ctxn[:], c1t[:, sl], op=SUB)


def _phase2(nc, tc, dram, qk_sb, v_sb, gtiles, vdefer):
    wo1, wo2, y = dram
    tri_sb, onescol_sb, onesrow_sb = gtiles
    with (
        tc.tile_pool(name="wop", bufs=1) as wopool,
        tc.tile_pool(name="ctx1p", bufs=2) as c1pool,
        tc.tile_pool(name="ctx2p", bufs=2) as c2pool,
        tc.tile_pool(name="cxs", bufs=2) as cxspool,
        tc.tile_pool(name="cxn", bufs=2) as cxnpool,
        tc.tile_pool(name="rrow", bufs=2) as rrpool,
        tc.tile_pool(name="pp", bufs=4) as ppool,
        tc.tile_pool(name="yp", bufs=3) as ypool,
        tc.tile_pool(name="pss", bufs=2, space="PSUM") as pss,
        tc.tile_pool(name="psm", bufs=1, space="PSUM") as psm,
        tc.tile_pool(name="psc", bufs=2, space="PSUM") as psc,
        tc.tile_pool(name="psy", bufs=3, space="PSUM") as psy,
    ):
        wo_sb = {}

        def load_wo():
            for hp in range(2):
                for lv, ten in ((1, wo1), (2, wo2)):
                    t_ = wopool.tile([P, 2, C], F8, tag=f"wo{hp}_{lv}")
                    nc.sync.dma_start(t_[:], ten[hp])
                    wo_sb[(hp, lv)] = t_

        ctx1 = {}   # (jb, hp) -> [P, 2, 512] fp8
        ctx2 = {}
        yrow = {}

        def outproj_thunk(jb, sub, ob):
            def run():
                tt = jb * 4 + sub
                ssl = slice(sub * P, (sub + 1) * P)
                osl = slice(ob * 512, (ob + 1) * 512)
                yps = psy.tile([P, 512], F32, tag="y", bufs=3)
                n = 0
                for hp in range(2):
                    for ct, wl in ((ctx1, 1), (ctx1, 2), (ctx2, 1)):
                        nc.tensor.matmul(
                            yps[:], ct[(jb, hp)][:, :, ssl],
                            wo_sb[(hp, wl)][:, :, osl],
                            start=(n == 0), stop=(n == 5), perf_mode=DR)
                        n += 1
                if ob == 0:
                    yrow[tt] = ypool.tile([P, T], BF, tag="ysb", bufs=2,
                                          name=f"yrow{tt}")
                y_sb = yrow[tt]
                if (sub + ob) % 2 == 0:
                    nc.vector.tensor_scalar_mul(y_sb[:, osl], yps[:], 1.0 / WS)
                else:
                    nc.scalar.mul(y_sb[:, osl], yps[:], 1.0 / WS)
                if jb == NB - 1 and sub == NB - 1:
                    nc.sync.dma_start(y[tt * P:(tt + 1) * P, osl],
                                      y_sb[:, osl])
                elif ob == NB - 1:
                    nc.sync.dma_start(y[tt * P:(tt + 1) * P, :], y_sb[:])
            return run

        xtail, wv_sb = vdefer

        def v_thunk(tt):
            def run():
                ssl = slice(tt * P - 3 * 512, tt * P - 3 * 512 + P)
                vacc = psy.tile([P, 512], F32, tag="y", bufs=3, name="vacc")
                n = 0
                for xl, wl in ((1, 1), (1, 2), (2, 1)):
                    for j in range(2, NPAIR):
                        nc.tensor.matmul(
                            vacc[:], xtail[(xl, j)][:, :, ssl], wv_sb[wl][:, j],
                            start=(n == 0), stop=(n == 17), perf_mode=DR)
                        n += 1
                nc.vector.tensor_tensor(v_sb[tt][:], v_sb[tt][:], vacc[:],
                                        op=ADD)
            return run

        pending = [v_thunk(tt) for tt in range(12, NTT)]
        ahead = {"pend": pending, "it": 0, "kint": 0}

        def interleave():
            ahead["it"] += 1
            if ahead["pend"] and ahead["kint"] and \
                    ahead["it"] % ahead["kint"] == 0:
                ahead["pend"].pop(0)()

        att_pools = (pss, psc, psm, ppool, rrpool)
        att_consts = (tri_sb, onescol_sb)
        nrm_pools = (None, rrpool, cxspool, cxnpool)
        nrm_consts = (onesrow_sb,)

        load_wo()
        xfin = [None]

        for jb in range(NB):
            if xfin[0] is not None:
                xfin[0]()
                xfin[0] = None

            nt = 4 * (jb + 1)
            ahead["it"] = 0
            ahead["kint"] = (HPC * nt) // len(pending) if pending else 0

            fin = None
            for h in range(HPC):
                ctx_ps, rinv_row = _attention_head(
                    nc, att_pools, qk_sb, v_sb, att_consts, jb, h, interleave,
                    head_start=fin)
                hp, sl = h // 2, h % 2
                if sl == 0:
                    ctx1[(jb, hp)] = c1pool.tile(
                        [P, 2, 512], F8, tag=f"c1_{hp}", bufs=2,
                        name=f"c1_{jb}_{hp}")
                    ctx2[(jb, hp)] = c2pool.tile(
                        [P, 2, 512], F8, tag=f"c2_{hp}", bufs=2,
                        name=f"c2_{jb}_{hp}")

                def fin(cp=ctx_ps, rr=rinv_row, c1t=ctx1[(jb, hp)],
                        c2t=ctx2[(jb, hp)], s=sl):
                    _normalize_ctx(nc, nrm_pools, nrm_consts, cp, rr,
                                   c1t, c2t, s)
            if jb + 1 < NB:
                xfin[0] = fin
            else:
                fin()

            while pending:
                pending.pop(0)()
            pending.extend(outproj_thunk(jb, sub, ob)
                           for sub in range(4) for ob in range(4))
            ahead["pend"] = pending

        while pending:
            pending.pop(0)()


def _build():
    nc = bacc.Bacc("TRN2", target_bir_lowering=False, debug=False,
                   num_devices=NCORES)
    x1 = nc.dram_tensor("x1", (NPAIR, P, 2, T), F8, kind="ExternalInput").ap()
    x2 = nc.dram_tensor("x2", (NPAIR, P, 2, T), F8, kind="ExternalInput").ap()
    wq1 = nc.dram_tensor("wq1", (HPC, P, NPAIR, 2, P), F8, kind="ExternalInput").ap()
    wq2 = nc.dram_tensor("wq2", (HPC, P, NPAIR, 2, P), F8, kind="ExternalInput").ap()
    wk1 = nc.dram_tensor("wk1", (HPC, P, NPAIR, 2, P), F8, kind="ExternalInput").ap()
    wk2 = nc.dram_tensor("wk2", (HPC, P, NPAIR, 2, P), F8, kind="ExternalInput").ap()
    wv1 = nc.dram_tensor("wv1", (P, NPAIR, 2, HPC * P), F8, kind="ExternalInput").ap()
    wv2 = nc.dram_tensor("wv2", (P, NPAIR, 2, HPC * P), F8, kind="ExternalInput").ap()
    wo1 = nc.dram_tensor("wo1", (2, P, 2, C), F8, kind="ExternalInput").ap()
    wo2 = nc.dram_tensor("wo2", (2, P, 2, C), F8, kind="ExternalInput").ap()
    cosT = nc.dram_tensor("cosT", (P, T), BF, kind="ExternalInput").ap()
    sinT = nc.dram_tensor("sinT", (P, T), BF, kind="ExternalInput").ap()
    tri = nc.dram_tensor("tri", (P, P), BF, kind="ExternalInput").ap()
    onescol = nc.dram_tensor("onescol", (P, 1), BF, kind="ExternalInput").ap()
    onesrow = nc.dram_tensor("onesrow", (1, P), BF, kind="ExternalInput").ap()
    y = nc.dram_tensor("y", (T, C), BF, kind="ExternalOutput").ap()

    with tile.TileContext(nc) as tc:
        with (
            tc.tile_pool(name="gconst", bufs=1) as gpool,
            tc.tile_pool(name="qkbuf", bufs=1) as qkpool,
            tc.tile_pool(name="vbuf", bufs=1) as vpool,
        ):
            tri_sb = gpool.tile([P, P], BF, tag="tri")
            onescol_sb = gpool.tile([P, 1], BF, tag="ocol")
            onesrow_sb = gpool.tile([1, P], BF, tag="orow")
            cos_sb = gpool.tile([P, T], BF, tag="cos")
            sin_sb = gpool.tile([P, T], BF, tag="sin")

            def const_loads():
                nc.sync.dma_start(cos_sb[:], cosT)
                nc.sync.dma_start(sin_sb[:], sinT)
                nc.sync.dma_start(tri_sb[:], tri)
                nc.sync.dma_start(onescol_sb[:], onescol)
                nc.sync.dma_start(onesrow_sb[:], onesrow)

            wvpool_cm = tc.tile_pool(name="wvp", bufs=1)
            wvpool = wvpool_cm.__enter__()
            xtpool_cm = tc.tile_pool(name="xtail", bufs=1)
            xtpool = xtpool_cm.__enter__()
            xtail = {}

            def xtail_loads():
                for j in range(2, NPAIR):
                    for lv, ten in ((1, x1), (2, x2)):
                        t_ = xtpool.tile([P, 2, 512], F8, tag=f"xt{lv}_{j}",
                                         name=f"xt{lv}_{j}")
                        nc.sync.dma_start(t_[:], ten[j][:, :, 3 * 512:])
                        xtail[(lv, j)] = t_

            spool_cm = tc.tile_pool(name="st", bufs=2)
            spool = spool_cm.__enter__()
            half = P // 2

            def rope_row(part, h):
                # whole-row rope for (part, h): the half-swap is 2 big DMAs
                # instead of 8 small ones (descriptor floor dominates small
                # transfers), and the mul/mul/add chain is 3 ops per row
                src = qk_sb[(part, h)]
                tmp = spool.tile([P, T], BF, tag="rt", bufs=2, name="rtmp")
                t1 = spool.tile([P, T], BF, tag="t1", bufs=2, name="rt1")
                t2 = spool.tile([P, T], BF, tag="t2", bufs=2, name="rt2")
                nc.sync.dma_start(tmp[0:half, :], src[half:P, :])
                nc.sync.dma_start(tmp[half:P, :], src[0:half, :])
                nc.gpsimd.tensor_tensor(t1[:], src[:], cos_sb[:], op=MULT)
                nc.vector.tensor_tensor(t2[:], tmp[:], sin_sb[:], op=MULT)
                nc.vector.tensor_tensor(src[:], t1[:], t2[:], op=ADD)

            qk_sb = {}
            for h in range(HPC):
                for part in ("q", "k"):
                    qk_sb[(part, h)] = qkpool.tile(
                        [P, T], BF, tag=f"{part}{h}",
                        name=f"{part}{h}_sb")
            v_sb = [vpool.tile([P, HPC * P], BF, tag=f"vb{i}", name=f"v{i}_sb")
                    for i in range(NTT)]

            wv_sb = _phase1(nc, tc, (x1, x2, wq1, wq2, wk1, wk2, wv1, wv2),
                            qk_sb, v_sb, const_loads, rope_row, wvpool,
                            xtail_loads)
            spool_cm.__exit__(None, None, None)
            _phase2(nc, tc, (wo1, wo2, y), qk_sb, v_sb,
                    (tri_sb, onescol_sb, onesrow_sb), (xtail, wv_sb))
            xtpool_cm.__exit__(None, None, None)
            wvpool_cm.__exit__(None, None, None)

    nc.compile()
    return nc


def _build_kernel():
    if "k" not in _CACHE:
        _CACHE["k"] = _build()
    return _CACHE["k"]


E4NP = ml_dtypes.float8_e4m3
BFNP = ml_dtypes.bfloat16


def _split8(a):
    a1 = np.asarray(a, np.float32).astype(E4NP)
    a2 = (np.asarray(a, np.float32) - a1.astype(np.float32)).astype(E4NP)
    return a1, a2


def prepare_in_maps(x, W_qkv, W_o, cos, sin):
    tri01 = (np.arange(P)[:, None] <= np.arange(P)[None, :]).astype(BFNP)
    onescol = np.full((P, 1), WS, dtype=np.float32).astype(BFNP)
    onesrow = np.ones((1, P), dtype=np.float32).astype(BFNP)
    cosT = np.ascontiguousarray(cos.T).astype(BFNP)
    sgn = np.where(np.arange(P) < P // 2, -1.0, 1.0).astype(np.float32)
    sinT = np.ascontiguousarray(sin.T * sgn[:, None]).astype(BFNP)

    # per-batch x fp8 pair chunks [NPAIR, P, 2, T]
    xq = {}
    for b in range(2):
        xT = np.ascontiguousarray(x[b].T)
        xs = _split8(xT)
        xq[b] = tuple(
            np.ascontiguousarray(
                a.reshape(NPAIR, 2, P, T).transpose(0, 2, 1, 3))
            for a in xs)

    def wqk_layout(a):   # [C, 512] -> (HPC, P, NPAIR, 2, P)
        return np.ascontiguousarray(
            a.reshape(NPAIR, 2, P, HPC, P).transpose(3, 2, 0, 1, 4))

    def wv_layout(a):    # [C, 512] -> (P, NPAIR, 2, HPC*P)
        return np.ascontiguousarray(
            a.reshape(NPAIR, 2, P, HPC * P).transpose(2, 0, 1, 3))

    in_maps = []
    for core in range(NCORES):
        b = core // 4
        hg0 = (core % 4) * HPC
        rows = slice(hg0 * P, (hg0 + HPC) * P)
        wq_r = WS * W_qkv[0 * C:1 * C][rows].T   # [C, 512]
        wk_r = WS * W_qkv[1 * C:2 * C][rows].T
        wv_r = WS * W_qkv[2 * C:3 * C][rows].T
        wq1, wq2 = (wqk_layout(a) for a in _split8(wq_r))
        wk1, wk2 = (wqk_layout(a) for a in _split8(wk_r))
        wv1, wv2 = (wv_layout(a) for a in _split8(wv_r))
        wo_r = WS * W_o[:, rows].T               # [512, C]
        wo1, wo2 = (
            np.ascontiguousarray(a.reshape(2, 2, P, C).transpose(0, 2, 1, 3))
            for a in _split8(wo_r))
        in_maps.append({
            "x1": xq[b][0], "x2": xq[b][1],
            "wq1": wq1, "wq2": wq2, "wk1": wk1, "wk2": wk2,
            "wv1": wv1, "wv2": wv2, "wo1": wo1, "wo2": wo2,
            "cosT": cosT, "sinT": sinT, "tri": tri01,
            "onescol": onescol, "onesrow": onesrow,
        })
    return in_maps


def gather(results, b_o):
    y = np.zeros((2, T, C), dtype=np.float32)
    for core in range(NCORES):
        y[core // 4] += np.asarray(results[core]["y"], dtype=np.float32)
    y += np.asarray(b_o, dtype=np.float32)[None, None, :]
    return y


def kernel(x, W_qkv, W_o, b_o, cos, sin):
    x = np.asarray(x, dtype=np.float32)
    W_qkv = np.asarray(W_qkv, dtype=np.float32)
    W_o = np.asarray(W_o, dtype=np.float32)
    cos = np.asarray(cos, dtype=np.float32)
    sin = np.asarray(sin, dtype=np.float32)
    nc = _build_kernel()
    in_maps = prepare_in_maps(x, W_qkv, W_o, cos, sin)
    res = run_bass_kernel_spmd(nc, in_maps, core_ids=list(range(NCORES)))
    return gather(res.results, b_o)


# revision 50
# speedup vs baseline: 1.0046x; 1.0046x over previous
"""Causal multi-head attention (RoPE) on 8 TRN2 NeuronCores.

Problem: x[2,2048,2048] -> qkv proj -> rope -> causal attention (16 heads,
head_dim 128) -> output proj + bias. Sharding: (batch, head-group) across the
8 cores - core c handles batch c//4 and heads 4*(c%4)..4*(c%4)+3. Each core
computes a partial output projection over its heads' channels; the host sums
the 4 partials per batch and adds b_o.

Mixed-precision pipeline (tolerance 2e-2; this lands ~3.5e-3):
  - QKV projection and output projection run in fp8 (e4m3) with a hi/lo
    3-term split (W1X1 + W1X2 + W2X1, weights pre-scaled by 64 into the
    e4m3 normal range) using DoubleRow matmuls: each instruction contracts
    2x128 rows at 0.5 cycles per output column - 2.67x the f32r rate for
    the same accuracy class.
  - Attention (scores, exp, AV) runs in bf16 (1 cyc/col, no 256-col floor,
    so causal narrowing works at 128-col granularity).
  - Softmax row-sums l use pt as the matmul *stationary* operand with a
    [128,1] ones column as the moving operand: cost 1 cycle per tile-chunk
    instead of N. 1/l is transposed back to row form via 4 tiny PE
    transposes + 4 K=1 broadcast matmuls.
  - All evictions/elementwise work spread across Pool/DVE/ACT to keep the
    sidecar engines under the PE roofline.

Layout: all matmuls keep contraction on partitions; q,k produced transposed
[d, tok], v natural [tok, (h,d)]; scores transposed s^T[tk, tq] so softmax
needs no transposes; ctx^T[d, tq] accumulates over tk tiles; outproj
contracts the 4 heads' channels as 2 DoubleRow head-pairs. The output
projection for block jb-1 is interleaved into block jb's attention inner
loops to fill the PE bubbles left by the exp dependency chain.
"""
import math

import numpy as np
import ml_dtypes

import concourse.bacc as bacc
import concourse.mybir as mybir
import concourse.tile as tile
from concourse.bass_utils import run_bass_kernel_spmd

P = 128           # partitions / head_dim
T = 2048          # context length
C = 2048          # d_model
NTT = T // P      # 16 token tiles
NB = T // 512     # 4 query blocks of 512
HPC = 4           # heads per core
NPAIR = C // 256  # 8 DoubleRow contraction pair-chunks
NCORES = 8
WS = 64.0         # fp8 weight pre-scale
SCALE = 1.0 / math.sqrt(P)
ESC = SCALE / (WS * WS)   # exp() scale: scores carry WS^2

F32 = mybir.dt.float32
BF = mybir.dt.bfloat16
F8 = mybir.dt.float8e4
EXP = mybir.ActivationFunctionType.Exp
MULT = mybir.AluOpType.mult
ADD = mybir.AluOpType.add
SUB = mybir.AluOpType.subtract
DR = mybir.MatmulPerfMode.DoubleRow

_CACHE = {}


def _phase1(nc, tc, dram, qk_sb, v_sb, const_loads, rope0, wvpool,
            xtail_loads):
    """QKV projection: fp8 hi/lo 3-term DoubleRow. Pass A covers pair-chunks
    0-1 (so PE starts ~4us in, DMA-paced), pass B covers 2-7; pass-B partials
    merge into bf16 SBUF via DVE adds. rope0(part, h) is invoked after each
    pass-B group so block-0 rope chains run during pass B."""
    x1, x2, wq1, wq2, wk1, wk2, wv1, wv2 = dram
    PASSES = (list(range(0, 2)), list(range(2, 8)))
    with (
        tc.tile_pool(name="xp", bufs=1) as xpool,
        tc.tile_pool(name="wp", bufs=1) as wpool,
        tc.tile_pool(name="psqk", bufs=4, space="PSUM") as psqk,
        tc.tile_pool(name="psv", bufs=3, space="PSUM") as psv,
    ):
        wten = {"q": (wq1, wq2), "k": (wk1, wk2)}
        groups = [(h, part) for h in range(HPC) for part in ("q", "k")]

        def load_w(gi, ps):
            h, part = groups[gi]
            js = PASSES[ps]
            jsl = slice(js[0], js[-1] + 1)
            w_sb = {}
            for lv in (1, 2):
                t_ = wpool.tile([P, len(js), 2, P], F8, tag=f"w{ps}_{gi}_{lv}",
                                bufs=1, name=f"w{part}{h}p{ps}_{lv}")
                nc.sync.dma_start(t_[:], wten[part][lv - 1][h][:, jsl])
                w_sb[lv] = t_
            return w_sb

        wq_pref = {(0, 0): load_w(0, 0)}
        xt = {}

        def load_x(js_):
            for j in js_:
                for lv, ten in ((1, x1), (2, x2)):
                    t_ = xpool.tile([P, 2, T], F8, tag=f"x{lv}_{j}",
                                    bufs=1, name=f"x{lv}_{j}")
                    nc.sync.dma_start(t_[:], ten[j])
                    xt[(lv, j)] = t_

        load_x(PASSES[0])
        for gi in range(1, len(groups)):
            wq_pref[(gi, 0)] = load_w(gi, 0)
        wv_sb = {}
        for lv, ten in ((1, wv1), (2, wv2)):
            t_ = wvpool.tile([P, NPAIR, 2, HPC * P], F8, tag=f"wv{lv}")
            nc.sync.dma_start(t_[:], ten)
            wv_sb[lv] = t_
        load_x(PASSES[1])
        const_loads()
        xtail_loads()
        for gi in range(len(groups)):
            wq_pref[(gi, 1)] = load_w(gi, 1)

        for ps in range(2):
            js = PASSES[ps]
            nmm = 3 * len(js)
            for gi, (h, part) in enumerate(groups):
                w_sb = wq_pref.pop((gi, ps))
                for nb in range(NB):
                    tsl = slice(nb * 512, (nb + 1) * 512)
                    acc = psqk.tile([P, 512], F32, tag="qk", bufs=4)
                    n = 0
                    order = ([(jj, t) for jj in range(len(js)) for t in range(3)]
                             if ps == 0 else
                             [(jj, t) for t in range(3) for jj in range(len(js))])
                    terms = ((1, 1), (1, 2), (2, 1))
                    for jj, t in order:
                        wl, xl = terms[t]
                        nc.tensor.matmul(
                            acc[:], w_sb[wl][:, jj], xt[(xl, js[jj])][:, :, tsl],
                            start=(n == 0), stop=(n == nmm - 1),
                            perf_mode=DR)
                        n += 1
                    dst = qk_sb[(part, h)]
                    if ps == 0:
                        nc.scalar.copy(dst[:, tsl], acc[:])
                    else:
                        nc.vector.tensor_tensor(dst[:, tsl], dst[:, tsl],
                                                acc[:], op=ADD)
                # two v token-tiles after each q/k group (the last four
                # pass-B tiles are deferred into block-0 attention)
                for tt in (2 * gi, 2 * gi + 1):
                    if ps == 1 and tt >= 12:
                        continue
                    ssl = slice(tt * P, (tt + 1) * P)
                    vacc = psv.tile([P, 512], F32, tag="v", bufs=3)
                    n = 0
                    for xl, wl in ((1, 1), (1, 2), (2, 1)):
                        for jj, j in enumerate(js):
                            nc.tensor.matmul(
                                vacc[:], xt[(xl, j)][:, :, ssl], wv_sb[wl][:, j],
                                start=(n == 0), stop=(n == nmm - 1),
                                perf_mode=DR)
                            n += 1
                    if ps == 0:
                        nc.scalar.copy(v_sb[tt][:], vacc[:])
                    else:
                        nc.vector.tensor_tensor(v_sb[tt][:], v_sb[tt][:],
                                                vacc[:], op=ADD)
                if ps == 1:
                    rope0(part, h)
    return wv_sb


def _attention_head(nc, pools, qk_sb, v_sb, consts, jb, h, interleave,
                    head_start=None):
    """Scores/exp/mask/AV/l for one (jb, h), with score pipelining and
    outproj interleave. head_start (the previous head's deferred
    normalize chain) is emitted after this head's first two scores so
    its PE/DVE ops hide behind fresh score work."""
    pss, psc, psl, ppool, rrpool = pools
    tri_sb, onescol_sb = consts
    qT = qk_sb[("q", h)]
    qsl = slice(jb * 512, (jb + 1) * 512)
    nt = 4 * (jb + 1)
    ctx_ps = psc.tile([P, 512], F32, tag="ctx", bufs=2)
    l_ps = psl.tile([1, 512], F32, tag="l", bufs=1)

    def score(i):
        r = i - 4 * jb
        c0 = max(0, r * P)
        osl = slice(c0, 512)
        sps = pss.tile([P, 512], F32, tag="s", bufs=2)
        kT = qk_sb[("k", h)]
        nc.tensor.matmul(sps[:, osl], kT[:, i * P:(i + 1) * P],
                         qT[:, jb * 512 + c0:(jb + 1) * 512],
                         start=True, stop=True)
        pt = ppool.tile([P, 512], BF, tag="pt", bufs=5)
        nc.scalar.activation(pt[:, osl], sps[:, osl], EXP, scale=ESC)
        if r >= 0:
            dsl = slice(r * P, (r + 1) * P)
            nc.gpsimd.tensor_tensor(pt[:, dsl], pt[:, dsl], tri_sb[:], op=MULT)
        return pt, c0

    ahead = 4
    queue = [score(i) for i in range(min(ahead, nt))]
    for i in range(nt):
        pt, c0 = queue.pop(0)
        if i + ahead < nt:
            queue.append(score(i + ahead))
        if i == 0 and head_start is not None:
            head_start()
        osl = slice(c0, 512)
        nc.tensor.matmul(ctx_ps[:, osl], v_sb[i][:, h * P:(h + 1) * P],
                         pt[:, osl], start=(i == 0), stop=(i == nt - 1))
        nc.tensor.matmul(l_ps[:, osl], onescol_sb[:], pt[:, osl],
                         start=(i == 0), stop=(i == nt - 1))
        interleave()
    rinv_row = rrpool.tile([1, 512], BF, tag="rr", bufs=2)
    with nc.allow_low_precision(reason="softmax 1/l bf16"):
        nc.vector.reciprocal(rinv_row[:], l_ps[:])
    return ctx_ps, rinv_row


def _normalize_ctx(nc, pools, consts, ctx_ps, rinv_row, c1t, c2t, sl):
    """broadcast 1/l -> normalize -> split ctx into fp8 hi/lo pair slots."""
    psb, rrpool, cxspool, cxnpool = pools
    onesrow_sb, = consts
    bps = rrpool.tile([P, 512], BF, tag="bb", bufs=2, name="bps_sb")
    nc.gpsimd.partition_broadcast(bps[:], rinv_row[:])
    cvt = cxspool.tile([P, 512], F32, tag="cvt", bufs=2)
    nc.vector.tensor_copy(cvt[:], ctx_ps[:])
    ctxn = cxnpool.tile([P, 512], F32, tag="cxn", bufs=2)
    nc.vector.tensor_tensor(ctxn[:], cvt[:], bps[:], op=MULT)
    nc.vector.tensor_copy(c1t[:, sl], ctxn[:])
    nc.vector.tensor_tensor(c2t[:, sl], ctxn[:], c1t[:, sl], op=SUB)


def _phase2(nc, tc, dram, qk_sb, v_sb, gtiles, vdefer):
    wo1, wo2, y = dram
    tri_sb, onescol_sb, onesrow_sb = gtiles
    with (
        tc.tile_pool(name="wop", bufs=1) as wopool,
        tc.tile_pool(name="ctx1p", bufs=2) as c1pool,
        tc.tile_pool(name="ctx2p", bufs=2) as c2pool,
        tc.tile_pool(name="cxs", bufs=2) as cxspool,
        tc.tile_pool(name="cxn", bufs=2) as cxnpool,
        tc.tile_pool(name="rrow", bufs=2) as rrpool,
        tc.tile_pool(name="pp", bufs=4) as ppool,
        tc.tile_pool(name="yp", bufs=3) as ypool,
        tc.tile_pool(name="pss", bufs=2, space="PSUM") as pss,
        tc.tile_pool(name="psm", bufs=1, space="PSUM") as psm,
        tc.tile_pool(name="psc", bufs=2, space="PSUM") as psc,
        tc.tile_pool(name="psy", bufs=3, space="PSUM") as psy,
    ):
        wo_sb = {}

        def load_wo():
            for hp in range(2):
                for lv, ten in ((1, wo1), (2, wo2)):
                    t_ = wopool.tile([P, 2, C], F8, tag=f"wo{hp}_{lv}")
                    nc.sync.dma_start(t_[:], ten[hp])
                    wo_sb[(hp, lv)] = t_

        ctx1 = {}   # (jb, hp) -> [P, 2, 512] fp8
        ctx2 = {}
        yrow = {}

        def outproj_thunk(jb, sub, ob):
            def run():
                tt = jb * 4 + sub
                ssl = slice(sub * P, (sub + 1) * P)
                osl = slice(ob * 512, (ob + 1) * 512)
                yps = psy.tile([P, 512], F32, tag="y", bufs=3)
                n = 0
                for hp in range(2):
                    for ct, wl in ((ctx1, 1), (ctx1, 2), (ctx2, 1)):
                        nc.tensor.matmul(
                            yps[:], ct[(jb, hp)][:, :, ssl],
                            wo_sb[(hp, wl)][:, :, osl],
                            start=(n == 0), stop=(n == 5), perf_mode=DR)
                        n += 1
                if ob == 0:
                    yrow[tt] = ypool.tile([P, T], BF, tag="ysb", bufs=3,
                                          name=f"yrow{tt}")
                y_sb = yrow[tt]
                if (sub + ob) % 2 == 0:
                    nc.vector.tensor_scalar_mul(y_sb[:, osl], yps[:], 1.0 / WS)
                else:
                    nc.scalar.mul(y_sb[:, osl], yps[:], 1.0 / WS)
                if jb == NB - 1 and sub == NB - 1:
                    nc.sync.dma_start(y[tt * P:(tt + 1) * P, osl],
                                      y_sb[:, osl])
                elif ob == NB - 1:
                    nc.sync.dma_start(y[tt * P:(tt + 1) * P, :], y_sb[:])
            return run

        xtail, wv_sb = vdefer

        def v_thunk(tt):
            def run():
                ssl = slice(tt * P - 3 * 512, tt * P - 3 * 512 + P)
                vacc = psy.tile([P, 512], F32, tag="y", bufs=3, name="vacc")
                n = 0
                for xl, wl in ((1, 1), (1, 2), (2, 1)):
                    for j in range(2, NPAIR):
                        nc.tensor.matmul(
                            vacc[:], xtail[(xl, j)][:, :, ssl], wv_sb[wl][:, j],
                            start=(n == 0), stop=(n == 17), perf_mode=DR)
                        n += 1
                nc.vector.tensor_tensor(v_sb[tt][:], v_sb[tt][:], vacc[:],
                                        op=ADD)
            return run

        pending = [v_thunk(tt) for tt in range(12, NTT)]
        ahead = {"pend": pending, "it": 0, "kint": 0}

        def interleave():
            ahead["it"] += 1
            if ahead["pend"] and ahead["kint"] and \
                    ahead["it"] % ahead["kint"] == 0:
                ahead["pend"].pop(0)()

        att_pools = (pss, psc, psm, ppool, rrpool)
        att_consts = (tri_sb, onescol_sb)
        nrm_pools = (None, rrpool, cxspool, cxnpool)
        nrm_consts = (onesrow_sb,)

        load_wo()
        xfin = [None]

        for jb in range(NB):
            if xfin[0] is not None:
                xfin[0]()
                xfin[0] = None

            nt = 4 * (jb + 1)
            ahead["it"] = 0
            ahead["kint"] = (HPC * nt) // len(pending) if pending else 0

            fin = None
            for h in range(HPC):
                ctx_ps, rinv_row = _attention_head(
                    nc, att_pools, qk_sb, v_sb, att_consts, jb, h, interleave,
                    head_start=fin)
                hp, sl = h // 2, h % 2
                if sl == 0:
                    ctx1[(jb, hp)] = c1pool.tile(
                        [P, 2, 512], F8, tag=f"c1_{hp}", bufs=2,
                        name=f"c1_{jb}_{hp}")
                    ctx2[(jb, hp)] = c2pool.tile(
                        [P, 2, 512], F8, tag=f"c2_{hp}", bufs=2,
                        name=f"c2_{jb}_{hp}")

                def fin(cp=ctx_ps, rr=rinv_row, c1t=ctx1[(jb, hp)],
                        c2t=ctx2[(jb, hp)], s=sl):
                    _normalize_ctx(nc, nrm_pools, nrm_consts, cp, rr,
                                   c1t, c2t, s)
            if jb + 1 < NB:
                xfin[0] = fin
            else:
                fin()

            while pending:
                pending.pop(0)()
            pending.extend(outproj_thunk(jb, sub, ob)
                           for sub in range(4) for ob in range(4))
            ahead["pend"] = pending

        while pending:
            pending.pop(0)()


def _build():
    nc = bacc.Bacc("TRN2", target_bir_lowering=False, debug=False,
                   num_devices=NCORES)
    x1 = nc.dram_tensor("x1", (NPAIR, P, 2, T), F8, kind="ExternalInput").ap()
    x2 = nc.dram_tensor("x2", (NPAIR, P, 2, T), F8, kind="ExternalInput").ap()
    wq1 = nc.dram_tensor("wq1", (HPC, P, NPAIR, 2, P), F8, kind="ExternalInput").ap()
    wq2 = nc.dram_tensor("wq2", (HPC, P, NPAIR, 2, P), F8, kind="ExternalInput").ap()
    wk1 = nc.dram_tensor("wk1", (HPC, P, NPAIR, 2, P), F8, kind="ExternalInput").ap()
    wk2 = nc.dram_tensor("wk2", (HPC, P, NPAIR, 2, P), F8, kind="ExternalInput").ap()
    wv1 = nc.dram_tensor("wv1", (P, NPAIR, 2, HPC * P), F8, kind="ExternalInput").ap()
    wv2 = nc.dram_tensor("wv2", (P, NPAIR, 2, HPC * P), F8, kind="ExternalInput").ap()
    wo1 = nc.dram_tensor("wo1", (2, P, 2, C), F8, kind="ExternalInput").ap()
    wo2 = nc.dram_tensor("wo2", (2, P, 2, C), F8, kind="ExternalInput").ap()
    cosT = nc.dram_tensor("cosT", (P, T), BF, kind="ExternalInput").ap()
    sinT = nc.dram_tensor("sinT", (P, T), BF, kind="ExternalInput").ap()
    tri = nc.dram_tensor("tri", (P, P), BF, kind="ExternalInput").ap()
    onescol = nc.dram_tensor("onescol", (P, 1), BF, kind="ExternalInput").ap()
    onesrow = nc.dram_tensor("onesrow", (1, P), BF, kind="ExternalInput").ap()
    y = nc.dram_tensor("y", (T, C), BF, kind="ExternalOutput").ap()

    with tile.TileContext(nc) as tc:
        with (
            tc.tile_pool(name="gconst", bufs=1) as gpool,
            tc.tile_pool(name="qkbuf", bufs=1) as qkpool,
            tc.tile_pool(name="vbuf", bufs=1) as vpool,
        ):
            tri_sb = gpool.tile([P, P], BF, tag="tri")
            onescol_sb = gpool.tile([P, 1], BF, tag="ocol")
            onesrow_sb = gpool.tile([1, P], BF, tag="orow")
            cos_sb = gpool.tile([P, T], BF, tag="cos")
            sin_sb = gpool.tile([P, T], BF, tag="sin")

            def const_loads():
                nc.sync.dma_start(cos_sb[:], cosT)
                nc.sync.dma_start(sin_sb[:], sinT)
                nc.sync.dma_start(tri_sb[:], tri)
                nc.sync.dma_start(onescol_sb[:], onescol)
                nc.sync.dma_start(onesrow_sb[:], onesrow)

            wvpool_cm = tc.tile_pool(name="wvp", bufs=1)
            wvpool = wvpool_cm.__enter__()
            xtpool_cm = tc.tile_pool(name="xtail", bufs=1)
            xtpool = xtpool_cm.__enter__()
            xtail = {}

            def xtail_loads():
                for j in range(2, NPAIR):
                    for lv, ten in ((1, x1), (2, x2)):
                        t_ = xtpool.tile([P, 2, 512], F8, tag=f"xt{lv}_{j}",
                                         name=f"xt{lv}_{j}")
                        nc.sync.dma_start(t_[:], ten[j][:, :, 3 * 512:])
                        xtail[(lv, j)] = t_

            spool_cm = tc.tile_pool(name="st", bufs=2)
            spool = spool_cm.__enter__()
            half = P // 2

            def rope_row(part, h):
                # whole-row rope for (part, h): the half-swap is 2 big DMAs
                # instead of 8 small ones (descriptor floor dominates small
                # transfers), and the mul/mul/add chain is 3 ops per row
                src = qk_sb[(part, h)]
                tmp = spool.tile([P, T], BF, tag="rt", bufs=2, name="rtmp")
                t1 = spool.tile([P, T], BF, tag="t1", bufs=2, name="rt1")
                t2 = spool.tile([P, T], BF, tag="t2", bufs=2, name="rt2")
                nc.sync.dma_start(tmp[0:half, :], src[half:P, :])
                nc.sync.dma_start(tmp[half:P, :], src[0:half, :])
                nc.gpsimd.tensor_tensor(t1[:], src[:], cos_sb[:], op=MULT)
                nc.vector.tensor_tensor(t2[:], tmp[:], sin_sb[:], op=MULT)
                nc.vector.tensor_tensor(src[:], t1[:], t2[:], op=ADD)

            qk_sb = {}
            for h in range(HPC):
                for part in ("q", "k"):
                    qk_sb[(part, h)] = qkpool.tile(
                        [P, T], BF, tag=f"{part}{h}",
                        name=f"{part}{h}_sb")
            v_sb = [vpool.tile([P, HPC * P], BF, tag=f"vb{i}", name=f"v{i}_sb")
                    for i in range(NTT)]

            wv_sb = _phase1(nc, tc, (x1, x2, wq1, wq2, wk1, wk2, wv1, wv2),
                            qk_sb, v_sb, const_loads, rope_row, wvpool,
                            xtail_loads)
            spool_cm.__exit__(None, None, None)
            _phase2(nc, tc, (wo1, wo2, y), qk_sb, v_sb,
                    (tri_sb, onescol_sb, onesrow_sb), (xtail, wv_sb))
            xtpool_cm.__exit__(None, None, None)
            wvpool_cm.__exit__(None, None, None)

    nc.compile()
    return nc


def _build_kernel():
    if "k" not in _CACHE:
        _CACHE["k"] = _build()
    return _CACHE["k"]


E4NP = ml_dtypes.float8_e4m3
BFNP = ml_dtypes.bfloat16


def _split8(a):
    a1 = np.asarray(a, np.float32).astype(E4NP)
    a2 = (np.asarray(a, np.float32) - a1.astype(np.float32)).astype(E4NP)
    return a1, a2


def prepare_in_maps(x, W_qkv, W_o, cos, sin):
    tri01 = (np.arange(P)[:, None] <= np.arange(P)[None, :]).astype(BFNP)
    onescol = np.full((P, 1), WS, dtype=np.float32).astype(BFNP)
    onesrow = np.ones((1, P), dtype=np.float32).astype(BFNP)
    cosT = np.ascontiguousarray(cos.T).astype(BFNP)
    sgn = np.where(np.arange(P) < P // 2, -1.0, 1.0).astype(np.float32)
    sinT = np.ascontiguousarray(sin.T * sgn[:, None]).astype(BFNP)

    # per-batch x fp8 pair chunks [NPAIR, P, 2, T]
    xq = {}
    for b in range(2):
        xT = np.ascontiguousarray(x[b].T)
        xs = _split8(xT)
        xq[b] = tuple(
            np.ascontiguousarray(
                a.reshape(NPAIR, 2, P, T).transpose(0, 2, 1, 3))
            for a in xs)

    def wqk_layout(a):   # [C, 512] -> (HPC, P, NPAIR, 2, P)
        return np.ascontiguousarray(
            a.reshape(NPAIR, 2, P, HPC, P).transpose(3, 2, 0, 1, 4))

    def wv_layout(a):    # [C, 512] -> (P, NPAIR, 2, HPC*P)
        return np.ascontiguousarray(
            a.reshape(NPAIR, 2, P, HPC * P).transpose(2, 0, 1, 3))

    in_maps = []
    for core in range(NCORES):
        b = core // 4
        hg0 = (core % 4) * HPC
        rows = slice(hg0 * P, (hg0 + HPC) * P)
        wq_r = WS * W_qkv[0 * C:1 * C][rows].T   # [C, 512]
        wk_r = WS * W_qkv[1 * C:2 * C][rows].T
        wv_r = WS * W_qkv[2 * C:3 * C][rows].T
        wq1, wq2 = (wqk_layout(a) for a in _split8(wq_r))
        wk1, wk2 = (wqk_layout(a) for a in _split8(wk_r))
        wv1, wv2 = (wv_layout(a) for a in _split8(wv_r))
        wo_r = WS * W_o[:, rows].T               # [512, C]
        wo1, wo2 = (
            np.ascontiguousarray(a.reshape(2, 2, P, C).transpose(0, 2, 1, 3))
            for a in _split8(wo_r))
        in_maps.append({
            "x1": xq[b][0], "x2": xq[b][1],
            "wq1": wq1, "wq2": wq2, "wk1": wk1, "wk2": wk2,
            "wv1": wv1, "wv2": wv2, "wo1": wo1, "wo2": wo2,
            "cosT": cosT, "sinT": sinT, "tri": tri01,
            "onescol": onescol, "onesrow": onesrow,
        })
    return in_maps


def gather(results, b_o):
    y = np.zeros((2, T, C), dtype=np.float32)
    for core in range(NCORES):
        y[core // 4] += np.asarray(results[core]["y"], dtype=np.float32)
    y += np.asarray(b_o, dtype=np.float32)[None, None, :]
    return y


def kernel(x, W_qkv, W_o, b_o, cos, sin):
    x = np.asarray(x, dtype=np.float32)
    W_qkv = np.asarray(W_qkv, dtype=np.float32)
    W_o = np.asarray(W_o, dtype=np.float32)
    cos = np.asarray(cos, dtype=np.float32)
    sin = np.asarray(sin, dtype=np.float32)
    nc = _build_kernel()
    in_maps = prepare_in_maps(x, W_qkv, W_o, cos, sin)
    res = run_bass_kernel_spmd(nc, in_maps, core_ids=list(range(NCORES)))
    return gather(res.results, b_o)


# revision 55
# speedup vs baseline: 1.0113x; 1.0066x over previous
"""Causal multi-head attention (RoPE) on 8 TRN2 NeuronCores.

Problem: x[2,2048,2048] -> qkv proj -> rope -> causal attention (16 heads,
head_dim 128) -> output proj + bias. Sharding: (batch, head-group) across the
8 cores - core c handles batch c//4 and heads 4*(c%4)..4*(c%4)+3. Each core
computes a partial output projection over its heads' channels; the host sums
the 4 partials per batch and adds b_o.

Mixed-precision pipeline (tolerance 2e-2; this lands ~3.5e-3):
  - QKV projection and output projection run in fp8 (e4m3) with a hi/lo
    3-term split (W1X1 + W1X2 + W2X1, weights pre-scaled by 64 into the
    e4m3 normal range) using DoubleRow matmuls: each instruction contracts
    2x128 rows at 0.5 cycles per output column - 2.67x the f32r rate for
    the same accuracy class.
  - Attention (scores, exp, AV) runs in bf16 (1 cyc/col, no 256-col floor,
    so causal narrowing works at 128-col granularity).
  - Softmax row-sums l use pt as the matmul *stationary* operand with a
    [128,1] ones column as the moving operand: cost 1 cycle per tile-chunk
    instead of N. 1/l is transposed back to row form via 4 tiny PE
    transposes + 4 K=1 broadcast matmuls.
  - All evictions/elementwise work spread across Pool/DVE/ACT to keep the
    sidecar engines under the PE roofline.

Layout: all matmuls keep contraction on partitions; q,k produced transposed
[d, tok], v natural [tok, (h,d)]; scores transposed s^T[tk, tq] so softmax
needs no transposes; ctx^T[d, tq] accumulates over tk tiles; outproj
contracts the 4 heads' channels as 2 DoubleRow head-pairs. The output
projection for block jb-1 is interleaved into block jb's attention inner
loops to fill the PE bubbles left by the exp dependency chain.
"""
import math

import numpy as np
import ml_dtypes

import concourse.bacc as bacc
import concourse.mybir as mybir
import concourse.tile as tile
from concourse.bass_utils import run_bass_kernel_spmd

P = 128           # partitions / head_dim
T = 2048          # context length
C = 2048          # d_model
NTT = T // P      # 16 token tiles
NB = T // 512     # 4 query blocks of 512
HPC = 4           # heads per core
NPAIR = C // 256  # 8 DoubleRow contraction pair-chunks
NCORES = 8
WS = 64.0         # fp8 weight pre-scale
SCALE = 1.0 / math.sqrt(P)
ESC = SCALE / (WS * WS)   # exp() scale: scores carry WS^2

F32 = mybir.dt.float32
BF = mybir.dt.bfloat16
F8 = mybir.dt.float8e4
EXP = mybir.ActivationFunctionType.Exp
MULT = mybir.AluOpType.mult
ADD = mybir.AluOpType.add
SUB = mybir.AluOpType.subtract
DR = mybir.MatmulPerfMode.DoubleRow

_CACHE = {}


def _phase1(nc, tc, dram, qk_sb, v_sb, const_loads, rope0, wvpool,
            xtail_loads):
    """QKV projection: fp8 hi/lo 3-term DoubleRow. Pass A covers pair-chunks
    0-1 (so PE starts ~4us in, DMA-paced), pass B covers 2-7; pass-B partials
    merge into bf16 SBUF via DVE adds. rope0(part, h) is invoked after each
    pass-B group so block-0 rope chains run during pass B."""
    x1, x2, wq1, wq2, wk1, wk2, wv1, wv2 = dram
    PASSES = (list(range(0, 2)), list(range(2, 8)))
    with (
        tc.tile_pool(name="xp", bufs=1) as xpool,
        tc.tile_pool(name="wp", bufs=1) as wpool,
        tc.tile_pool(name="psqk", bufs=4, space="PSUM") as psqk,
        tc.tile_pool(name="psv", bufs=3, space="PSUM") as psv,
    ):
        wten = {"q": (wq1, wq2), "k": (wk1, wk2)}
        groups = [(h, part) for h in range(HPC) for part in ("q", "k")]

        def load_w(gi, ps):
            h, part = groups[gi]
            js = PASSES[ps]
            jsl = slice(js[0], js[-1] + 1)
            w_sb = {}
            for lv in (1, 2):
                t_ = wpool.tile([P, len(js), 2, P], F8, tag=f"w{ps}_{gi}_{lv}",
                                bufs=1, name=f"w{part}{h}p{ps}_{lv}")
                nc.sync.dma_start(t_[:], wten[part][lv - 1][h][:, jsl])
                w_sb[lv] = t_
            return w_sb

        wq_pref = {(0, 0): load_w(0, 0)}
        xt = {}

        def load_x(js_):
            for j in js_:
                for lv, ten in ((1, x1), (2, x2)):
                    t_ = xpool.tile([P, 2, T], F8, tag=f"x{lv}_{j}",
                                    bufs=1, name=f"x{lv}_{j}")
                    nc.sync.dma_start(t_[:], ten[j])
                    xt[(lv, j)] = t_

        load_x(PASSES[0])
        for gi in range(1, len(groups)):
            wq_pref[(gi, 0)] = load_w(gi, 0)
        wv_sb = {}
        for lv, ten in ((1, wv1), (2, wv2)):
            t_ = wvpool.tile([P, NPAIR, 2, HPC * P], F8, tag=f"wv{lv}")
            nc.sync.dma_start(t_[:], ten)
            wv_sb[lv] = t_
        load_x(PASSES[1])
        const_loads()
        xtail_loads()
        for gi in range(len(groups)):
            wq_pref[(gi, 1)] = load_w(gi, 1)

        for ps in range(2):
            js = PASSES[ps]
            nmm = 3 * len(js)
            for gi, (h, part) in enumerate(groups):
                w_sb = wq_pref.pop((gi, ps))
                for nb in range(NB):
                    tsl = slice(nb * 512, (nb + 1) * 512)
                    acc = psqk.tile([P, 512], F32, tag="qk", bufs=4)
                    n = 0
                    order = ([(jj, t) for jj in range(len(js)) for t in range(3)]
                             if ps == 0 else
                             [(jj, t) for t in range(3) for jj in range(len(js))])
                    terms = ((1, 1), (1, 2), (2, 1))
                    for jj, t in order:
                        wl, xl = terms[t]
                        nc.tensor.matmul(
                            acc[:], w_sb[wl][:, jj], xt[(xl, js[jj])][:, :, tsl],
                            start=(n == 0), stop=(n == nmm - 1),
                            perf_mode=DR)
                        n += 1
                    dst = qk_sb[(part, h)]
                    if ps == 0:
                        nc.scalar.copy(dst[:, tsl], acc[:])
                    else:
                        nc.vector.tensor_tensor(dst[:, tsl], dst[:, tsl],
                                                acc[:], op=ADD)
                # two v token-tiles after each q/k group (the last four
                # pass-B tiles are deferred into block-0 attention)
                for tt in (2 * gi, 2 * gi + 1):
                    if ps == 1 and tt >= 12:
                        continue
                    ssl = slice(tt * P, (tt + 1) * P)
                    vacc = psv.tile([P, 512], F32, tag="v", bufs=3)
                    n = 0
                    for xl, wl in ((1, 1), (1, 2), (2, 1)):
                        for jj, j in enumerate(js):
                            nc.tensor.matmul(
                                vacc[:], xt[(xl, j)][:, :, ssl], wv_sb[wl][:, j],
                                start=(n == 0), stop=(n == nmm - 1),
                                perf_mode=DR)
                            n += 1
                    if ps == 0:
                        nc.scalar.copy(v_sb[tt][:], vacc[:])
                    else:
                        nc.vector.tensor_tensor(v_sb[tt][:], v_sb[tt][:],
                                                vacc[:], op=ADD)
                if ps == 1:
                    rope0(part, h)
    return wv_sb


def _attention_head(nc, pools, qk_sb, v_sb, consts, jb, h, interleave,
                    head_start=None):
    """Scores/exp/mask/AV/l for one (jb, h), with score pipelining and
    outproj interleave. head_start (the previous head's deferred
    normalize chain) is emitted after this head's first two scores so
    its PE/DVE ops hide behind fresh score work."""
    pss, psc, psl, ppool, rrpool = pools
    tri_sb, onescol_sb = consts
    qT = qk_sb[("q", h)]
    qsl = slice(jb * 512, (jb + 1) * 512)
    nt = 4 * (jb + 1)
    ctx_ps = psc.tile([P, 512], F32, tag="ctx", bufs=2)
    l_ps = psl.tile([1, 512], F32, tag="l", bufs=1)

    def score(i):
        r = i - 4 * jb
        c0 = max(0, r * P)
        osl = slice(c0, 512)
        sps = pss.tile([P, 512], F32, tag="s", bufs=2)
        kT = qk_sb[("k", h)]
        nc.tensor.matmul(sps[:, osl], kT[:, i * P:(i + 1) * P],
                         qT[:, jb * 512 + c0:(jb + 1) * 512],
                         start=True, stop=True)
        pt = ppool.tile([P, 512], BF, tag="pt", bufs=5)
        nc.scalar.activation(pt[:, osl], sps[:, osl], EXP, scale=ESC)
        if r >= 0:
            dsl = slice(r * P, (r + 1) * P)
            nc.gpsimd.tensor_tensor(pt[:, dsl], pt[:, dsl], tri_sb[:], op=MULT)
        return pt, c0

    ahead = 6
    queue = [score(i) for i in range(min(ahead, nt))]
    for i in range(nt):
        pt, c0 = queue.pop(0)
        if i + ahead < nt:
            queue.append(score(i + ahead))
        if i == 0 and head_start is not None:
            head_start()
        osl = slice(c0, 512)
        nc.tensor.matmul(ctx_ps[:, osl], v_sb[i][:, h * P:(h + 1) * P],
                         pt[:, osl], start=(i == 0), stop=(i == nt - 1))
        nc.tensor.matmul(l_ps[:, osl], onescol_sb[:], pt[:, osl],
                         start=(i == 0), stop=(i == nt - 1))
        interleave()
    rinv_row = rrpool.tile([1, 512], BF, tag="rr", bufs=2)
    with nc.allow_low_precision(reason="softmax 1/l bf16"):
        nc.vector.reciprocal(rinv_row[:], l_ps[:])
    return ctx_ps, rinv_row


def _normalize_ctx(nc, pools, consts, ctx_ps, rinv_row, c1t, c2t, sl):
    """broadcast 1/l -> normalize -> split ctx into fp8 hi/lo pair slots."""
    psb, rrpool, cxnpool = pools
    onesrow_sb, = consts
    bps = rrpool.tile([P, 512], BF, tag="bb", bufs=2, name="bps_sb")
    nc.gpsimd.partition_broadcast(bps[:], rinv_row[:])
    # bps lives in SBUF, so DVE can read the PSUM ctx directly (one PSUM
    # operand per op): the fp8 hi part and the f32 normalized ctx are
    # independent products of the same operands, then lo = f32 - hi.
    ctxn = cxnpool.tile([P, 512], F32, tag="cxn", bufs=2)
    nc.vector.tensor_tensor(ctxn[:], ctx_ps[:], bps[:], op=MULT)
    nc.vector.tensor_copy(c1t[:, sl], ctxn[:])
    nc.vector.tensor_tensor(c2t[:, sl], ctxn[:], c1t[:, sl], op=SUB)


def _phase2(nc, tc, dram, qk_sb, v_sb, gtiles, vdefer):
    wo1, wo2, y = dram
    tri_sb, onescol_sb, onesrow_sb = gtiles
    with (
        tc.tile_pool(name="wop", bufs=1) as wopool,
        tc.tile_pool(name="ctx1p", bufs=2) as c1pool,
        tc.tile_pool(name="ctx2p", bufs=2) as c2pool,
        tc.tile_pool(name="cxn", bufs=2) as cxnpool,
        tc.tile_pool(name="rrow", bufs=2) as rrpool,
        tc.tile_pool(name="pp", bufs=4) as ppool,
        tc.tile_pool(name="yp", bufs=3) as ypool,
        tc.tile_pool(name="pss", bufs=2, space="PSUM") as pss,
        tc.tile_pool(name="psm", bufs=1, space="PSUM") as psm,
        tc.tile_pool(name="psc", bufs=2, space="PSUM") as psc,
        tc.tile_pool(name="psy", bufs=3, space="PSUM") as psy,
    ):
        wo_sb = {}

        def load_wo():
            for hp in range(2):
                for lv, ten in ((1, wo1), (2, wo2)):
                    t_ = wopool.tile([P, 2, C], F8, tag=f"wo{hp}_{lv}")
                    nc.sync.dma_start(t_[:], ten[hp])
                    wo_sb[(hp, lv)] = t_

        ctx1 = {}   # (jb, hp) -> [P, 2, 512] fp8
        ctx2 = {}
        yrow = {}

        def outproj_thunk(jb, sub, ob):
            def run():
                tt = jb * 4 + sub
                ssl = slice(sub * P, (sub + 1) * P)
                osl = slice(ob * 512, (ob + 1) * 512)
                yps = psy.tile([P, 512], F32, tag="y", bufs=3)
                n = 0
                for hp in range(2):
                    for ct, wl in ((ctx1, 1), (ctx1, 2), (ctx2, 1)):
                        nc.tensor.matmul(
                            yps[:], ct[(jb, hp)][:, :, ssl],
                            wo_sb[(hp, wl)][:, :, osl],
                            start=(n == 0), stop=(n == 5), perf_mode=DR)
                        n += 1
                if ob == 0:
                    yrow[tt] = ypool.tile([P, T], BF, tag="ysb", bufs=3,
                                          name=f"yrow{tt}")
                y_sb = yrow[tt]
                if (sub + ob) % 2 == 0:
                    nc.vector.tensor_scalar_mul(y_sb[:, osl], yps[:], 1.0 / WS)
                else:
                    nc.scalar.mul(y_sb[:, osl], yps[:], 1.0 / WS)
                if jb == NB - 1 and sub == NB - 1:
                    nc.sync.dma_start(y[tt * P:(tt + 1) * P, osl],
                                      y_sb[:, osl])
                elif ob == NB - 1:
                    nc.sync.dma_start(y[tt * P:(tt + 1) * P, :], y_sb[:])
            return run

        xtail, wv_sb = vdefer

        def v_thunk(tt):
            def run():
                ssl = slice(tt * P - 3 * 512, tt * P - 3 * 512 + P)
                vacc = psy.tile([P, 512], F32, tag="y", bufs=3, name="vacc")
                n = 0
                for xl, wl in ((1, 1), (1, 2), (2, 1)):
                    for j in range(2, NPAIR):
                        nc.tensor.matmul(
                            vacc[:], xtail[(xl, j)][:, :, ssl], wv_sb[wl][:, j],
                            start=(n == 0), stop=(n == 17), perf_mode=DR)
                        n += 1
                nc.vector.tensor_tensor(v_sb[tt][:], v_sb[tt][:], vacc[:],
                                        op=ADD)
            return run

        pending = [v_thunk(tt) for tt in range(12, NTT)]
        ahead = {"pend": pending, "it": 0, "kint": 0}

        def interleave():
            ahead["it"] += 1
            if ahead["pend"] and ahead["kint"] and \
                    ahead["it"] % ahead["kint"] == 0:
                ahead["pend"].pop(0)()

        att_pools = (pss, psc, psm, ppool, rrpool)
        att_consts = (tri_sb, onescol_sb)
        nrm_pools = (None, rrpool, cxnpool)
        nrm_consts = (onesrow_sb,)

        load_wo()
        xfin = [None]

        for jb in range(NB):
            if xfin[0] is not None:
                xfin[0]()
                xfin[0] = None

            nt = 4 * (jb + 1)
            ahead["it"] = 0
            ahead["kint"] = (HPC * nt) // len(pending) if pending else 0

            fin = None
            for h in range(HPC):
                ctx_ps, rinv_row = _attention_head(
                    nc, att_pools, qk_sb, v_sb, att_consts, jb, h, interleave,
                    head_start=fin)
                hp, sl = h // 2, h % 2
                if sl == 0:
                    ctx1[(jb, hp)] = c1pool.tile(
                        [P, 2, 512], F8, tag=f"c1_{hp}", bufs=2,
                        name=f"c1_{jb}_{hp}")
                    ctx2[(jb, hp)] = c2pool.tile(
                        [P, 2, 512], F8, tag=f"c2_{hp}", bufs=2,
                        name=f"c2_{jb}_{hp}")

                def fin(cp=ctx_ps, rr=rinv_row, c1t=ctx1[(jb, hp)],
                        c2t=ctx2[(jb, hp)], s=sl):
                    _normalize_ctx(nc, nrm_pools, nrm_consts, cp, rr,
                                   c1t, c2t, s)
            if jb + 1 < NB:
                xfin[0] = fin
            else:
                fin()

            while pending:
                pending.pop(0)()
            pending.extend(outproj_thunk(jb, sub, ob)
                           for sub in range(4) for ob in range(4))
            ahead["pend"] = pending

        while pending:
            pending.pop(0)()


def _build():
    nc = bacc.Bacc("TRN2", target_bir_lowering=False, debug=False,
                   num_devices=NCORES)
    x1 = nc.dram_tensor("x1", (NPAIR, P, 2, T), F8, kind="ExternalInput").ap()
    x2 = nc.dram_tensor("x2", (NPAIR, P, 2, T), F8, kind="ExternalInput").ap()
    wq1 = nc.dram_tensor("wq1", (HPC, P, NPAIR, 2, P), F8, kind="ExternalInput").ap()
    wq2 = nc.dram_tensor("wq2", (HPC, P, NPAIR, 2, P), F8, kind="ExternalInput").ap()
    wk1 = nc.dram_tensor("wk1", (HPC, P, NPAIR, 2, P), F8, kind="ExternalInput").ap()
    wk2 = nc.dram_tensor("wk2", (HPC, P, NPAIR, 2, P), F8, kind="ExternalInput").ap()
    wv1 = nc.dram_tensor("wv1", (P, NPAIR, 2, HPC * P), F8, kind="ExternalInput").ap()
    wv2 = nc.dram_tensor("wv2", (P, NPAIR, 2, HPC * P), F8, kind="ExternalInput").ap()
    wo1 = nc.dram_tensor("wo1", (2, P, 2, C), F8, kind="ExternalInput").ap()
    wo2 = nc.dram_tensor("wo2", (2, P, 2, C), F8, kind="ExternalInput").ap()
    cosT = nc.dram_tensor("cosT", (P, T), BF, kind="ExternalInput").ap()
    sinT = nc.dram_tensor("sinT", (P, T), BF, kind="ExternalInput").ap()
    tri = nc.dram_tensor("tri", (P, P), BF, kind="ExternalInput").ap()
    onescol = nc.dram_tensor("onescol", (P, 1), BF, kind="ExternalInput").ap()
    onesrow = nc.dram_tensor("onesrow", (1, P), BF, kind="ExternalInput").ap()
    y = nc.dram_tensor("y", (T, C), BF, kind="ExternalOutput").ap()

    with tile.TileContext(nc) as tc:
        with (
            tc.tile_pool(name="gconst", bufs=1) as gpool,
            tc.tile_pool(name="qkbuf", bufs=1) as qkpool,
            tc.tile_pool(name="vbuf", bufs=1) as vpool,
        ):
            tri_sb = gpool.tile([P, P], BF, tag="tri")
            onescol_sb = gpool.tile([P, 1], BF, tag="ocol")
            onesrow_sb = gpool.tile([1, P], BF, tag="orow")
            cos_sb = gpool.tile([P, T], BF, tag="cos")
            sin_sb = gpool.tile([P, T], BF, tag="sin")

            def const_loads():
                nc.sync.dma_start(cos_sb[:], cosT)
                nc.sync.dma_start(sin_sb[:], sinT)
                nc.sync.dma_start(tri_sb[:], tri)
                nc.sync.dma_start(onescol_sb[:], onescol)
                nc.sync.dma_start(onesrow_sb[:], onesrow)

            wvpool_cm = tc.tile_pool(name="wvp", bufs=1)
            wvpool = wvpool_cm.__enter__()
            xtpool_cm = tc.tile_pool(name="xtail", bufs=1)
            xtpool = xtpool_cm.__enter__()
            xtail = {}

            def xtail_loads():
                for j in range(2, NPAIR):
                    for lv, ten in ((1, x1), (2, x2)):
                        t_ = xtpool.tile([P, 2, 512], F8, tag=f"xt{lv}_{j}",
                                         name=f"xt{lv}_{j}")
                        nc.sync.dma_start(t_[:], ten[j][:, :, 3 * 512:])
                        xtail[(lv, j)] = t_

            spool_cm = tc.tile_pool(name="st", bufs=2)
            spool = spool_cm.__enter__()
            half = P // 2

            def rope_row(part, h):
                # whole-row rope for (part, h): the half-swap is 2 big DMAs
                # instead of 8 small ones (descriptor floor dominates small
                # transfers), and the mul/mul/add chain is 3 ops per row
                src = qk_sb[(part, h)]
                tmp = spool.tile([P, T], BF, tag="rt", bufs=2, name="rtmp")
                t1 = spool.tile([P, T], BF, tag="t1", bufs=2, name="rt1")
                t2 = spool.tile([P, T], BF, tag="t2", bufs=2, name="rt2")
                nc.sync.dma_start(tmp[0:half, :], src[half:P, :])
                nc.sync.dma_start(tmp[half:P, :], src[0:half, :])
                nc.gpsimd.tensor_tensor(t1[:], src[:], cos_sb[:], op=MULT)
                nc.vector.tensor_tensor(t2[:], tmp[:], sin_sb[:], op=MULT)
                nc.vector.tensor_tensor(src[:], t1[:], t2[:], op=ADD)

            qk_sb = {}
            for h in range(HPC):
                for part in ("q", "k"):
                    qk_sb[(part, h)] = qkpool.tile(
                        [P, T], BF, tag=f"{part}{h}",
                        name=f"{part}{h}_sb")
            v_sb = [vpool.tile([P, HPC * P], BF, tag=f"vb{i}", name=f"v{i}_sb")
                    for i in range(NTT)]

            wv_sb = _phase1(nc, tc, (x1, x2, wq1, wq2, wk1, wk2, wv1, wv2),
                            qk_sb, v_sb, const_loads, rope_row, wvpool,
                            xtail_loads)
            spool_cm.__exit__(None, None, None)
            _phase2(nc, tc, (wo1, wo2, y), qk_sb, v_sb,
                    (tri_sb, onescol_sb, onesrow_sb), (xtail, wv_sb))
            xtpool_cm.__exit__(None, None, None)
            wvpool_cm.__exit__(None, None, None)

    nc.compile()
    return nc


def _build_kernel():
    if "k" not in _CACHE:
        _CACHE["k"] = _build()
    return _CACHE["k"]


E4NP = ml_dtypes.float8_e4m3
BFNP = ml_dtypes.bfloat16


def _split8(a):
    a1 = np.asarray(a, np.float32).astype(E4NP)
    a2 = (np.asarray(a, np.float32) - a1.astype(np.float32)).astype(E4NP)
    return a1, a2


def prepare_in_maps(x, W_qkv, W_o, cos, sin):
    tri01 = (np.arange(P)[:, None] <= np.arange(P)[None, :]).astype(BFNP)
    onescol = np.full((P, 1), WS, dtype=np.float32).astype(BFNP)
    onesrow = np.ones((1, P), dtype=np.float32).astype(BFNP)
    cosT = np.ascontiguousarray(cos.T).astype(BFNP)
    sgn = np.where(np.arange(P) < P // 2, -1.0, 1.0).astype(np.float32)
    sinT = np.ascontiguousarray(sin.T * sgn[:, None]).astype(BFNP)

    # per-batch x fp8 pair chunks [NPAIR, P, 2, T]
    xq = {}
    for b in range(2):
        xT = np.ascontiguousarray(x[b].T)
        xs = _split8(xT)
        xq[b] = tuple(
            np.ascontiguousarray(
                a.reshape(NPAIR, 2, P, T).transpose(0, 2, 1, 3))
            for a in xs)

    def wqk_layout(a):   # [C, 512] -> (HPC, P, NPAIR, 2, P)
        return np.ascontiguousarray(
            a.reshape(NPAIR, 2, P, HPC, P).transpose(3, 2, 0, 1, 4))

    def wv_layout(a):    # [C, 512] -> (P, NPAIR, 2, HPC*P)
        return np.ascontiguousarray(
            a.reshape(NPAIR, 2, P, HPC * P).transpose(2, 0, 1, 3))

    in_maps = []
    for core in range(NCORES):
        b = core // 4
        hg0 = (core % 4) * HPC
        rows = slice(hg0 * P, (hg0 + HPC) * P)
        wq_r = WS * W_qkv[0 * C:1 * C][rows].T   # [C, 512]
        wk_r = WS * W_qkv[1 * C:2 * C][rows].T
        wv_r = WS * W_qkv[2 * C:3 * C][rows].T
        wq1, wq2 = (wqk_layout(a) for a in _split8(wq_r))
        wk1, wk2 = (wqk_layout(a) for a in _split8(wk_r))
        wv1, wv2 = (wv_layout(a) for a in _split8(wv_r))
        wo_r = WS * W_o[:, rows].T               # [512, C]
        wo1, wo2 = (
            np.ascontiguousarray(a.reshape(2, 2, P, C).transpose(0, 2, 1, 3))
            for a in _split8(wo_r))
        in_maps.append({
            "x1": xq[b][0], "x2": xq[b][1],
            "wq1": wq1, "wq2": wq2, "wk1": wk1, "wk2": wk2,
            "wv1": wv1, "wv2": wv2, "wo1": wo1, "wo2": wo2,
            "cosT": cosT, "sinT": sinT, "tri": tri01,
            "onescol": onescol, "onesrow": onesrow,
        })
    return in_maps


def gather(results, b_o):
    y = np.zeros((2, T, C), dtype=np.float32)
    for core in range(NCORES):
        y[core // 4] += np.asarray(results[core]["y"], dtype=np.float32)
    y += np.asarray(b_o, dtype=np.float32)[None, None, :]
    return y


def kernel(x, W_qkv, W_o, b_o, cos, sin):
    x = np.asarray(x, dtype=np.float32)
    W_qkv = np.asarray(W_qkv, dtype=np.float32)
    W_o = np.asarray(W_o, dtype=np.float32)
    cos = np.asarray(cos, dtype=np.float32)
    sin = np.asarray(sin, dtype=np.float32)
    nc = _build_kernel()
    in_maps = prepare_in_maps(x, W_qkv, W_o, cos, sin)
    res = run_bass_kernel_spmd(nc, in_maps, core_ids=list(range(NCORES)))
    return gather(res.results, b_o)


# revision 57
# speedup vs baseline: 1.0129x; 1.0017x over previous
"""Causal multi-head attention (RoPE) on 8 TRN2 NeuronCores.

Problem: x[2,2048,2048] -> qkv proj -> rope -> causal attention (16 heads,
head_dim 128) -> output proj + bias. Sharding: (batch, head-group) across the
8 cores - core c handles batch c//4 and heads 4*(c%4)..4*(c%4)+3. Each core
computes a partial output projection over its heads' channels; the host sums
the 4 partials per batch and adds b_o.

Mixed-precision pipeline (tolerance 2e-2; this lands ~3.5e-3):
  - QKV projection and output projection run in fp8 (e4m3) with a hi/lo
    3-term split (W1X1 + W1X2 + W2X1, weights pre-scaled by 64 into the
    e4m3 normal range) using DoubleRow matmuls: each instruction contracts
    2x128 rows at 0.5 cycles per output column - 2.67x the f32r rate for
    the same accuracy class.
  - Attention (scores, exp, AV) runs in bf16 (1 cyc/col, no 256-col floor,
    so causal narrowing works at 128-col granularity).
  - Softmax row-sums l use pt as the matmul *stationary* operand with a
    [128,1] ones column as the moving operand: cost 1 cycle per tile-chunk
    instead of N. 1/l is transposed back to row form via 4 tiny PE
    transposes + 4 K=1 broadcast matmuls.
  - All evictions/elementwise work spread across Pool/DVE/ACT to keep the
    sidecar engines under the PE roofline.

Layout: all matmuls keep contraction on partitions; q,k produced transposed
[d, tok], v natural [tok, (h,d)]; scores transposed s^T[tk, tq] so softmax
needs no transposes; ctx^T[d, tq] accumulates over tk tiles; outproj
contracts the 4 heads' channels as 2 DoubleRow head-pairs. The output
projection for block jb-1 is interleaved into block jb's attention inner
loops to fill the PE bubbles left by the exp dependency chain.
"""
import math

import numpy as np
import ml_dtypes

import concourse.bacc as bacc
import concourse.mybir as mybir
import concourse.tile as tile
from concourse.bass_utils import run_bass_kernel_spmd

P = 128           # partitions / head_dim
T = 2048          # context length
C = 2048          # d_model
NTT = T // P      # 16 token tiles
NB = T // 512     # 4 query blocks of 512
HPC = 4           # heads per core
NPAIR = C // 256  # 8 DoubleRow contraction pair-chunks
NCORES = 8
WS = 64.0         # fp8 weight pre-scale
SCALE = 1.0 / math.sqrt(P)
ESC = SCALE / (WS * WS)   # exp() scale: scores carry WS^2

F32 = mybir.dt.float32
BF = mybir.dt.bfloat16
F8 = mybir.dt.float8e4
EXP = mybir.ActivationFunctionType.Exp
MULT = mybir.AluOpType.mult
ADD = mybir.AluOpType.add
SUB = mybir.AluOpType.subtract
DR = mybir.MatmulPerfMode.DoubleRow

_CACHE = {}


def _phase1(nc, tc, dram, qk_sb, v_sb, const_loads, rope0, wvpool,
            xtail_loads):
    """QKV projection: fp8 hi/lo 3-term DoubleRow. Pass A covers pair-chunks
    0-1 (so PE starts ~4us in, DMA-paced), pass B covers 2-7; pass-B partials
    merge into bf16 SBUF via DVE adds. rope0(part, h) is invoked after each
    pass-B group so block-0 rope chains run during pass B."""
    x1, x2, wq1, wq2, wk1, wk2, wv1, wv2 = dram
    PASSES = (list(range(0, 2)), list(range(2, 8)))
    with (
        tc.tile_pool(name="xp", bufs=1) as xpool,
        tc.tile_pool(name="wp", bufs=1) as wpool,
        tc.tile_pool(name="psqk", bufs=4, space="PSUM") as psqk,
        tc.tile_pool(name="psv", bufs=3, space="PSUM") as psv,
    ):
        wten = {"q": (wq1, wq2), "k": (wk1, wk2)}
        groups = [(h, part) for h in range(HPC) for part in ("q", "k")]

        def load_w(gi, ps):
            h, part = groups[gi]
            js = PASSES[ps]
            jsl = slice(js[0], js[-1] + 1)
            w_sb = {}
            for lv in (1, 2):
                t_ = wpool.tile([P, len(js), 2, P], F8, tag=f"w{ps}_{gi}_{lv}",
                                bufs=1, name=f"w{part}{h}p{ps}_{lv}")
                nc.sync.dma_start(t_[:], wten[part][lv - 1][h][:, jsl])
                w_sb[lv] = t_
            return w_sb

        wq_pref = {(0, 0): load_w(0, 0)}
        xt = {}

        def load_x(js_):
            for j in js_:
                for lv, ten in ((1, x1), (2, x2)):
                    t_ = xpool.tile([P, 2, T], F8, tag=f"x{lv}_{j}",
                                    bufs=1, name=f"x{lv}_{j}")
                    nc.sync.dma_start(t_[:], ten[j])
                    xt[(lv, j)] = t_

        load_x(PASSES[0])
        for gi in range(1, len(groups)):
            wq_pref[(gi, 0)] = load_w(gi, 0)
        wv_sb = {}
        for lv, ten in ((1, wv1), (2, wv2)):
            t_ = wvpool.tile([P, NPAIR, 2, HPC * P], F8, tag=f"wv{lv}")
            nc.sync.dma_start(t_[:], ten)
            wv_sb[lv] = t_
        load_x(PASSES[1])
        const_loads()
        xtail_loads()
        for gi in range(len(groups)):
            wq_pref[(gi, 1)] = load_w(gi, 1)

        for ps in range(2):
            js = PASSES[ps]
            nmm = 3 * len(js)
            for gi, (h, part) in enumerate(groups):
                w_sb = wq_pref.pop((gi, ps))
                for nb in range(NB):
                    tsl = slice(nb * 512, (nb + 1) * 512)
                    acc = psqk.tile([P, 512], F32, tag="qk", bufs=4)
                    n = 0
                    order = ([(jj, t) for jj in range(len(js)) for t in range(3)]
                             if ps == 0 else
                             [(jj, t) for t in range(3) for jj in range(len(js))])
                    terms = ((1, 1), (1, 2), (2, 1))
                    for jj, t in order:
                        wl, xl = terms[t]
                        nc.tensor.matmul(
                            acc[:], w_sb[wl][:, jj], xt[(xl, js[jj])][:, :, tsl],
                            start=(n == 0), stop=(n == nmm - 1),
                            perf_mode=DR)
                        n += 1
                    dst = qk_sb[(part, h)]
                    if ps == 0:
                        nc.scalar.copy(dst[:, tsl], acc[:])
                    else:
                        nc.vector.tensor_tensor(dst[:, tsl], dst[:, tsl],
                                                acc[:], op=ADD)
                # two v token-tiles after each q/k group (the last four
                # pass-B tiles are deferred into block-0 attention)
                for tt in (2 * gi, 2 * gi + 1):
                    if ps == 1 and tt >= 12:
                        continue
                    ssl = slice(tt * P, (tt + 1) * P)
                    vacc = psv.tile([P, 512], F32, tag="v", bufs=3)
                    n = 0
                    for xl, wl in ((1, 1), (1, 2), (2, 1)):
                        for jj, j in enumerate(js):
                            nc.tensor.matmul(
                                vacc[:], xt[(xl, j)][:, :, ssl], wv_sb[wl][:, j],
                                start=(n == 0), stop=(n == nmm - 1),
                                perf_mode=DR)
                            n += 1
                    if ps == 0:
                        nc.scalar.copy(v_sb[tt][:], vacc[:])
                    else:
                        nc.vector.tensor_tensor(v_sb[tt][:], v_sb[tt][:],
                                                vacc[:], op=ADD)
                if ps == 1:
                    rope0(part, h)
    return wv_sb


def _attention_head(nc, pools, qk_sb, v_sb, consts, jb, h, interleave,
                    head_start=None):
    """Scores/exp/mask/AV/l for one (jb, h), with score pipelining and
    outproj interleave. head_start (the previous head's deferred
    normalize chain) is emitted after this head's first two scores so
    its PE/DVE ops hide behind fresh score work."""
    pss, psc, psl, ppool, rrpool = pools
    tri_sb, onescol_sb = consts
    qT = qk_sb[("q", h)]
    qsl = slice(jb * 512, (jb + 1) * 512)
    nt = 4 * (jb + 1)
    ctx_ps = psc.tile([P, 512], F32, tag="ctx", bufs=2)
    l_ps = psl.tile([1, 512], F32, tag="l", bufs=1)

    def score(i):
        r = i - 4 * jb
        c0 = max(0, r * P)
        osl = slice(c0, 512)
        sps = pss.tile([P, 512], F32, tag="s", bufs=2)
        kT = qk_sb[("k", h)]
        nc.tensor.matmul(sps[:, osl], kT[:, i * P:(i + 1) * P],
                         qT[:, jb * 512 + c0:(jb + 1) * 512],
                         start=True, stop=True)
        pt = ppool.tile([P, 512], BF, tag="pt", bufs=5)
        nc.scalar.activation(pt[:, osl], sps[:, osl], EXP, scale=ESC)
        if r >= 0:
            dsl = slice(r * P, (r + 1) * P)
            nc.gpsimd.tensor_tensor(pt[:, dsl], pt[:, dsl], tri_sb[:], op=MULT)
        return pt, c0

    ahead = 6
    queue = [score(i) for i in range(min(ahead, nt))]
    for i in range(nt):
        pt, c0 = queue.pop(0)
        if i + ahead < nt:
            queue.append(score(i + ahead))
        if i == 0 and head_start is not None:
            head_start()
        osl = slice(c0, 512)
        nc.tensor.matmul(ctx_ps[:, osl], v_sb[i][:, h * P:(h + 1) * P],
                         pt[:, osl], start=(i == 0), stop=(i == nt - 1))
        nc.tensor.matmul(l_ps[:, osl], onescol_sb[:], pt[:, osl],
                         start=(i == 0), stop=(i == nt - 1))
        interleave()
    rinv_row = rrpool.tile([1, 512], BF, tag="rr", bufs=2)
    with nc.allow_low_precision(reason="softmax 1/l bf16"):
        nc.vector.reciprocal(rinv_row[:], l_ps[:])
    return ctx_ps, rinv_row


def _normalize_ctx(nc, pools, consts, ctx_ps, rinv_row, c1t, c2t, sl):
    """broadcast 1/l -> normalize -> split ctx into fp8 hi/lo pair slots."""
    psb, rrpool, cxnpool = pools
    onesrow_sb, = consts
    bps = rrpool.tile([P, 512], BF, tag="bb", bufs=2, name="bps_sb")
    nc.gpsimd.partition_broadcast(bps[:], rinv_row[:])
    # bps lives in SBUF, so DVE can read the PSUM ctx directly (one PSUM
    # operand per op): the fp8 hi part and the f32 normalized ctx are
    # independent products of the same operands, then lo = f32 - hi.
    ctxn = cxnpool.tile([P, 512], F32, tag="cxn", bufs=2)
    nc.vector.tensor_tensor(ctxn[:], ctx_ps[:], bps[:], op=MULT)
    nc.vector.tensor_copy(c1t[:, sl], ctxn[:])
    nc.vector.tensor_tensor(c2t[:, sl], ctxn[:], c1t[:, sl], op=SUB)


def _phase2(nc, tc, dram, qk_sb, v_sb, gtiles, vdefer):
    wo1, wo2, y = dram
    tri_sb, onescol_sb, onesrow_sb = gtiles
    with (
        tc.tile_pool(name="wop", bufs=1) as wopool,
        tc.tile_pool(name="ctx1p", bufs=2) as c1pool,
        tc.tile_pool(name="ctx2p", bufs=2) as c2pool,
        tc.tile_pool(name="cxn", bufs=2) as cxnpool,
        tc.tile_pool(name="rrow", bufs=2) as rrpool,
        tc.tile_pool(name="pp", bufs=4) as ppool,
        tc.tile_pool(name="yp", bufs=3) as ypool,
        tc.tile_pool(name="pss", bufs=2, space="PSUM") as pss,
        tc.tile_pool(name="psm", bufs=1, space="PSUM") as psm,
        tc.tile_pool(name="psc", bufs=2, space="PSUM") as psc,
        tc.tile_pool(name="psy", bufs=3, space="PSUM") as psy,
    ):
        wo_sb = {}

        def load_wo():
            for hp in range(2):
                for lv, ten in ((1, wo1), (2, wo2)):
                    t_ = wopool.tile([P, 2, C], F8, tag=f"wo{hp}_{lv}")
                    nc.sync.dma_start(t_[:], ten[hp])
                    wo_sb[(hp, lv)] = t_

        ctx1 = {}   # (jb, hp) -> [P, 2, 512] fp8
        ctx2 = {}
        yrow = {}

        def outproj_thunk(jb, sub, ob):
            # two pops per (sub, ob): 3 matmuls each, sharing one open
            # PSUM accumulation - finer interleave granularity
            box = {}

            def run_a():
                yps = psy.tile([P, 512], F32, tag="y", bufs=3, name="yps")
                box["yps"] = yps
                ssl = slice(sub * P, (sub + 1) * P)
                osl = slice(ob * 512, (ob + 1) * 512)
                for n, (ct, wl) in enumerate(((ctx1, 1), (ctx1, 2), (ctx2, 1))):
                    nc.tensor.matmul(
                        yps[:], ct[(jb, 0)][:, :, ssl],
                        wo_sb[(0, wl)][:, :, osl],
                        start=(n == 0), stop=False, perf_mode=DR)

            def run():
                tt = jb * 4 + sub
                ssl = slice(sub * P, (sub + 1) * P)
                osl = slice(ob * 512, (ob + 1) * 512)
                yps = box["yps"]
                for n, (ct, wl) in enumerate(((ctx1, 1), (ctx1, 2), (ctx2, 1))):
                    nc.tensor.matmul(
                        yps[:], ct[(jb, 1)][:, :, ssl],
                        wo_sb[(1, wl)][:, :, osl],
                        start=False, stop=(n == 2), perf_mode=DR)
                if ob == 0:
                    yrow[tt] = ypool.tile([P, T], BF, tag="ysb", bufs=3,
                                          name=f"yrow{tt}")
                y_sb = yrow[tt]
                if (sub + ob) % 2 == 0:
                    nc.vector.tensor_scalar_mul(y_sb[:, osl], yps[:], 1.0 / WS)
                else:
                    nc.scalar.mul(y_sb[:, osl], yps[:], 1.0 / WS)
                if jb == NB - 1 and sub == NB - 1:
                    nc.sync.dma_start(y[tt * P:(tt + 1) * P, osl],
                                      y_sb[:, osl])
                elif ob == NB - 1:
                    nc.sync.dma_start(y[tt * P:(tt + 1) * P, :], y_sb[:])
            return run_a, run

        xtail, wv_sb = vdefer

        def v_thunk(tt):
            def run():
                ssl = slice(tt * P - 3 * 512, tt * P - 3 * 512 + P)
                vacc = psy.tile([P, 512], F32, tag="y", bufs=3, name="vacc")
                n = 0
                for xl, wl in ((1, 1), (1, 2), (2, 1)):
                    for j in range(2, NPAIR):
                        nc.tensor.matmul(
                            vacc[:], xtail[(xl, j)][:, :, ssl], wv_sb[wl][:, j],
                            start=(n == 0), stop=(n == 17), perf_mode=DR)
                        n += 1
                nc.vector.tensor_tensor(v_sb[tt][:], v_sb[tt][:], vacc[:],
                                        op=ADD)
            return run

        pending = [v_thunk(tt) for tt in range(12, NTT)]
        ahead = {"pend": pending, "it": 0, "kint": 0}

        def interleave():
            ahead["it"] += 1
            if ahead["pend"] and ahead["kint"] and \
                    ahead["it"] % ahead["kint"] == 0:
                ahead["pend"].pop(0)()

        att_pools = (pss, psc, psm, ppool, rrpool)
        att_consts = (tri_sb, onescol_sb)
        nrm_pools = (None, rrpool, cxnpool)
        nrm_consts = (onesrow_sb,)

        load_wo()
        xfin = [None]

        for jb in range(NB):
            if xfin[0] is not None:
                xfin[0]()
                xfin[0] = None

            nt = 4 * (jb + 1)
            ahead["it"] = 0
            ahead["kint"] = (HPC * nt) // len(pending) if pending else 0

            fin = None
            for h in range(HPC):
                ctx_ps, rinv_row = _attention_head(
                    nc, att_pools, qk_sb, v_sb, att_consts, jb, h, interleave,
                    head_start=fin)
                hp, sl = h // 2, h % 2
                if sl == 0:
                    ctx1[(jb, hp)] = c1pool.tile(
                        [P, 2, 512], F8, tag=f"c1_{hp}", bufs=2,
                        name=f"c1_{jb}_{hp}")
                    ctx2[(jb, hp)] = c2pool.tile(
                        [P, 2, 512], F8, tag=f"c2_{hp}", bufs=2,
                        name=f"c2_{jb}_{hp}")

                def fin(cp=ctx_ps, rr=rinv_row, c1t=ctx1[(jb, hp)],
                        c2t=ctx2[(jb, hp)], s=sl):
                    _normalize_ctx(nc, nrm_pools, nrm_consts, cp, rr,
                                   c1t, c2t, s)
            if jb + 1 < NB:
                xfin[0] = fin
            else:
                fin()

            while pending:
                pending.pop(0)()
            for sub in range(4):
                for ob in range(4):
                    pending.extend(outproj_thunk(jb, sub, ob))
            ahead["pend"] = pending

        while pending:
            pending.pop(0)()


def _build():
    nc = bacc.Bacc("TRN2", target_bir_lowering=False, debug=False,
                   num_devices=NCORES)
    x1 = nc.dram_tensor("x1", (NPAIR, P, 2, T), F8, kind="ExternalInput").ap()
    x2 = nc.dram_tensor("x2", (NPAIR, P, 2, T), F8, kind="ExternalInput").ap()
    wq1 = nc.dram_tensor("wq1", (HPC, P, NPAIR, 2, P), F8, kind="ExternalInput").ap()
    wq2 = nc.dram_tensor("wq2", (HPC, P, NPAIR, 2, P), F8, kind="ExternalInput").ap()
    wk1 = nc.dram_tensor("wk1", (HPC, P, NPAIR, 2, P), F8, kind="ExternalInput").ap()
    wk2 = nc.dram_tensor("wk2", (HPC, P, NPAIR, 2, P), F8, kind="ExternalInput").ap()
    wv1 = nc.dram_tensor("wv1", (P, NPAIR, 2, HPC * P), F8, kind="ExternalInput").ap()
    wv2 = nc.dram_tensor("wv2", (P, NPAIR, 2, HPC * P), F8, kind="ExternalInput").ap()
    wo1 = nc.dram_tensor("wo1", (2, P, 2, C), F8, kind="ExternalInput").ap()
    wo2 = nc.dram_tensor("wo2", (2, P, 2, C), F8, kind="ExternalInput").ap()
    cosT = nc.dram_tensor("cosT", (P, T), BF, kind="ExternalInput").ap()
    sinT = nc.dram_tensor("sinT", (P, T), BF, kind="ExternalInput").ap()
    tri = nc.dram_tensor("tri", (P, P), BF, kind="ExternalInput").ap()
    onescol = nc.dram_tensor("onescol", (P, 1), BF, kind="ExternalInput").ap()
    onesrow = nc.dram_tensor("onesrow", (1, P), BF, kind="ExternalInput").ap()
    y = nc.dram_tensor("y", (T, C), BF, kind="ExternalOutput").ap()

    with tile.TileContext(nc) as tc:
        with (
            tc.tile_pool(name="gconst", bufs=1) as gpool,
            tc.tile_pool(name="qkbuf", bufs=1) as qkpool,
            tc.tile_pool(name="vbuf", bufs=1) as vpool,
        ):
            tri_sb = gpool.tile([P, P], BF, tag="tri")
            onescol_sb = gpool.tile([P, 1], BF, tag="ocol")
            onesrow_sb = gpool.tile([1, P], BF, tag="orow")
            cos_sb = gpool.tile([P, T], BF, tag="cos")
            sin_sb = gpool.tile([P, T], BF, tag="sin")

            def const_loads():
                nc.sync.dma_start(cos_sb[:], cosT)
                nc.sync.dma_start(sin_sb[:], sinT)
                nc.sync.dma_start(tri_sb[:], tri)
                nc.sync.dma_start(onescol_sb[:], onescol)
                nc.sync.dma_start(onesrow_sb[:], onesrow)

            wvpool_cm = tc.tile_pool(name="wvp", bufs=1)
            wvpool = wvpool_cm.__enter__()
            xtpool_cm = tc.tile_pool(name="xtail", bufs=1)
            xtpool = xtpool_cm.__enter__()
            xtail = {}

            def xtail_loads():
                for j in range(2, NPAIR):
                    for lv, ten in ((1, x1), (2, x2)):
                        t_ = xtpool.tile([P, 2, 512], F8, tag=f"xt{lv}_{j}",
                                         name=f"xt{lv}_{j}")
                        nc.sync.dma_start(t_[:], ten[j][:, :, 3 * 512:])
                        xtail[(lv, j)] = t_

            spool_cm = tc.tile_pool(name="st", bufs=2)
            spool = spool_cm.__enter__()
            half = P // 2

            def rope_row(part, h):
                # whole-row rope for (part, h): the half-swap is 2 big DMAs
                # instead of 8 small ones (descriptor floor dominates small
                # transfers), and the mul/mul/add chain is 3 ops per row
                src = qk_sb[(part, h)]
                tmp = spool.tile([P, T], BF, tag="rt", bufs=2, name="rtmp")
                t1 = spool.tile([P, T], BF, tag="t1", bufs=2, name="rt1")
                t2 = spool.tile([P, T], BF, tag="t2", bufs=2, name="rt2")
                nc.sync.dma_start(tmp[0:half, :], src[half:P, :])
                nc.sync.dma_start(tmp[half:P, :], src[0:half, :])
                nc.gpsimd.tensor_tensor(t1[:], src[:], cos_sb[:], op=MULT)
                nc.vector.tensor_tensor(t2[:], tmp[:], sin_sb[:], op=MULT)
                nc.vector.tensor_tensor(src[:], t1[:], t2[:], op=ADD)

            qk_sb = {}
            for h in range(HPC):
                for part in ("q", "k"):
                    qk_sb[(part, h)] = qkpool.tile(
                        [P, T], BF, tag=f"{part}{h}",
                        name=f"{part}{h}_sb")
            v_sb = [vpool.tile([P, HPC * P], BF, tag=f"vb{i}", name=f"v{i}_sb")
                    for i in range(NTT)]

            wv_sb = _phase1(nc, tc, (x1, x2, wq1, wq2, wk1, wk2, wv1, wv2),
                            qk_sb, v_sb, const_loads, rope_row, wvpool,
                            xtail_loads)
            spool_cm.__exit__(None, None, None)
            _phase2(nc, tc, (wo1, wo2, y), qk_sb, v_sb,
                    (tri_sb, onescol_sb, onesrow_sb), (xtail, wv_sb))
            xtpool_cm.__exit__(None, None, None)
            wvpool_cm.__exit__(None, None, None)

    nc.compile()
    return nc


def _build_kernel():
    if "k" not in _CACHE:
        _CACHE["k"] = _build()
    return _CACHE["k"]


E4NP = ml_dtypes.float8_e4m3
BFNP = ml_dtypes.bfloat16


def _split8(a):
    a1 = np.asarray(a, np.float32).astype(E4NP)
    a2 = (np.asarray(a, np.float32) - a1.astype(np.float32)).astype(E4NP)
    return a1, a2


def prepare_in_maps(x, W_qkv, W_o, cos, sin):
    tri01 = (np.arange(P)[:, None] <= np.arange(P)[None, :]).astype(BFNP)
    onescol = np.full((P, 1), WS, dtype=np.float32).astype(BFNP)
    onesrow = np.ones((1, P), dtype=np.float32).astype(BFNP)
    cosT = np.ascontiguousarray(cos.T).astype(BFNP)
    sgn = np.where(np.arange(P) < P // 2, -1.0, 1.0).astype(np.float32)
    sinT = np.ascontiguousarray(sin.T * sgn[:, None]).astype(BFNP)

    # per-batch x fp8 pair chunks [NPAIR, P, 2, T]
    xq = {}
    for b in range(2):
        xT = np.ascontiguousarray(x[b].T)
        xs = _split8(xT)
        xq[b] = tuple(
            np.ascontiguousarray(
                a.reshape(NPAIR, 2, P, T).transpose(0, 2, 1, 3))
            for a in xs)

    def wqk_layout(a):   # [C, 512] -> (HPC, P, NPAIR, 2, P)
        return np.ascontiguousarray(
            a.reshape(NPAIR, 2, P, HPC, P).transpose(3, 2, 0, 1, 4))

    def wv_layout(a):    # [C, 512] -> (P, NPAIR, 2, HPC*P)
        return np.ascontiguousarray(
            a.reshape(NPAIR, 2, P, HPC * P).transpose(2, 0, 1, 3))

    in_maps = []
    for core in range(NCORES):
        b = core // 4
        hg0 = (core % 4) * HPC
        rows = slice(hg0 * P, (hg0 + HPC) * P)
        wq_r = WS * W_qkv[0 * C:1 * C][rows].T   # [C, 512]
        wk_r = WS * W_qkv[1 * C:2 * C][rows].T
        wv_r = WS * W_qkv[2 * C:3 * C][rows].T
        wq1, wq2 = (wqk_layout(a) for a in _split8(wq_r))
        wk1, wk2 = (wqk_layout(a) for a in _split8(wk_r))
        wv1, wv2 = (wv_layout(a) for a in _split8(wv_r))
        wo_r = WS * W_o[:, rows].T               # [512, C]
        wo1, wo2 = (
            np.ascontiguousarray(a.reshape(2, 2, P, C).transpose(0, 2, 1, 3))
            for a in _split8(wo_r))
        in_maps.append({
            "x1": xq[b][0], "x2": xq[b][1],
            "wq1": wq1, "wq2": wq2, "wk1": wk1, "wk2": wk2,
            "wv1": wv1, "wv2": wv2, "wo1": wo1, "wo2": wo2,
            "cosT": cosT, "sinT": sinT, "tri": tri01,
            "onescol": onescol, "onesrow": onesrow,
        })
    return in_maps


def gather(results, b_o):
    y = np.zeros((2, T, C), dtype=np.float32)
    for core in range(NCORES):
        y[core // 4] += np.asarray(results[core]["y"], dtype=np.float32)
    y += np.asarray(b_o, dtype=np.float32)[None, None, :]
    return y


def kernel(x, W_qkv, W_o, b_o, cos, sin):
    x = np.asarray(x, dtype=np.float32)
    W_qkv = np.asarray(W_qkv, dtype=np.float32)
    W_o = np.asarray(W_o, dtype=np.float32)
    cos = np.asarray(cos, dtype=np.float32)
    sin = np.asarray(sin, dtype=np.float32)
    nc = _build_kernel()
    in_maps = prepare_in_maps(x, W_qkv, W_o, cos, sin)
    res = run_bass_kernel_spmd(nc, in_maps, core_ids=list(range(NCORES)))
    return gather(res.results, b_o)


# revision 60
# speedup vs baseline: 1.0133x; 1.0004x over previous
"""Causal multi-head attention (RoPE) on 8 TRN2 NeuronCores.

Problem: x[2,2048,2048] -> qkv proj -> rope -> causal attention (16 heads,
head_dim 128) -> output proj + bias. Sharding: (batch, head-group) across the
8 cores - core c handles batch c//4 and heads 4*(c%4)..4*(c%4)+3. Each core
computes a partial output projection over its heads' channels; the host sums
the 4 partials per batch and adds b_o.

Mixed-precision pipeline (tolerance 2e-2; this lands ~3.5e-3):
  - QKV projection and output projection run in fp8 (e4m3) with a hi/lo
    3-term split (W1X1 + W1X2 + W2X1, weights pre-scaled by 64 into the
    e4m3 normal range) using DoubleRow matmuls: each instruction contracts
    2x128 rows at 0.5 cycles per output column - 2.67x the f32r rate for
    the same accuracy class.
  - Attention (scores, exp, AV) runs in bf16 (1 cyc/col, no 256-col floor,
    so causal narrowing works at 128-col granularity).
  - Softmax row-sums l use pt as the matmul *stationary* operand with a
    [128,1] ones column as the moving operand: cost 1 cycle per tile-chunk
    instead of N. 1/l is transposed back to row form via 4 tiny PE
    transposes + 4 K=1 broadcast matmuls.
  - All evictions/elementwise work spread across Pool/DVE/ACT to keep the
    sidecar engines under the PE roofline.

Layout: all matmuls keep contraction on partitions; q,k produced transposed
[d, tok], v natural [tok, (h,d)]; scores transposed s^T[tk, tq] so softmax
needs no transposes; ctx^T[d, tq] accumulates over tk tiles; outproj
contracts the 4 heads' channels as 2 DoubleRow head-pairs. The output
projection for block jb-1 is interleaved into block jb's attention inner
loops to fill the PE bubbles left by the exp dependency chain.
"""
import math

import numpy as np
import ml_dtypes

import concourse.bacc as bacc
import concourse.mybir as mybir
import concourse.tile as tile
from concourse.bass_utils import run_bass_kernel_spmd

P = 128           # partitions / head_dim
T = 2048          # context length
C = 2048          # d_model
NTT = T // P      # 16 token tiles
NB = T // 512     # 4 query blocks of 512
HPC = 4           # heads per core
NPAIR = C // 256  # 8 DoubleRow contraction pair-chunks
NCORES = 8
WS = 64.0         # fp8 weight pre-scale
SCALE = 1.0 / math.sqrt(P)
ESC = SCALE / (WS * WS)   # exp() scale: scores carry WS^2

F32 = mybir.dt.float32
BF = mybir.dt.bfloat16
F8 = mybir.dt.float8e4
EXP = mybir.ActivationFunctionType.Exp
MULT = mybir.AluOpType.mult
ADD = mybir.AluOpType.add
SUB = mybir.AluOpType.subtract
DR = mybir.MatmulPerfMode.DoubleRow

_CACHE = {}


def _phase1(nc, tc, dram, qk_sb, v_sb, const_loads, rope0, wvpool,
            xtail_loads):
    """QKV projection: fp8 hi/lo 3-term DoubleRow. Pass A covers pair-chunks
    0-1 (so PE starts ~4us in, DMA-paced), pass B covers 2-7; pass-B partials
    merge into bf16 SBUF via DVE adds. rope0(part, h) is invoked after each
    pass-B group so block-0 rope chains run during pass B."""
    x1, x2, wq1, wq2, wk1, wk2, wv1, wv2 = dram
    PASSES = (list(range(0, 2)), list(range(2, 8)))
    with (
        tc.tile_pool(name="xp", bufs=1) as xpool,
        tc.tile_pool(name="wp", bufs=1) as wpool,
        tc.tile_pool(name="psqk", bufs=4, space="PSUM") as psqk,
        tc.tile_pool(name="psv", bufs=3, space="PSUM") as psv,
    ):
        wten = {"q": (wq1, wq2), "k": (wk1, wk2)}
        groups = [(h, part) for h in range(HPC) for part in ("q", "k")]

        def load_w(gi, ps):
            h, part = groups[gi]
            js = PASSES[ps]
            jsl = slice(js[0], js[-1] + 1)
            w_sb = {}
            for lv in (1, 2):
                t_ = wpool.tile([P, len(js), 2, P], F8, tag=f"w{ps}_{gi}_{lv}",
                                bufs=1, name=f"w{part}{h}p{ps}_{lv}")
                nc.sync.dma_start(t_[:], wten[part][lv - 1][h][:, jsl])
                w_sb[lv] = t_
            return w_sb

        wq_pref = {(0, 0): load_w(0, 0)}
        xt = {}

        def load_x(js_):
            for j in js_:
                for lv, ten in ((1, x1), (2, x2)):
                    t_ = xpool.tile([P, 2, T], F8, tag=f"x{lv}_{j}",
                                    bufs=1, name=f"x{lv}_{j}")
                    nc.sync.dma_start(t_[:], ten[j])
                    xt[(lv, j)] = t_

        load_x(PASSES[0])
        for gi in range(1, len(groups)):
            wq_pref[(gi, 0)] = load_w(gi, 0)
        wv_sb = {}
        for lv, ten in ((1, wv1), (2, wv2)):
            t_ = wvpool.tile([P, NPAIR, 2, HPC * P], F8, tag=f"wv{lv}")
            nc.sync.dma_start(t_[:], ten)
            wv_sb[lv] = t_
        load_x(PASSES[1])
        const_loads()
        xtail_loads()
        for gi in range(len(groups)):
            wq_pref[(gi, 1)] = load_w(gi, 1)

        for ps in range(2):
            js = PASSES[ps]
            nmm = 3 * len(js)
            for gi, (h, part) in enumerate(groups):
                w_sb = wq_pref.pop((gi, ps))
                for nb in range(NB):
                    tsl = slice(nb * 512, (nb + 1) * 512)
                    acc = psqk.tile([P, 512], F32, tag="qk", bufs=4)
                    n = 0
                    order = ([(jj, t) for jj in range(len(js)) for t in range(3)]
                             if ps == 0 else
                             [(jj, t) for t in range(3) for jj in range(len(js))])
                    terms = ((1, 1), (1, 2), (2, 1))
                    for jj, t in order:
                        wl, xl = terms[t]
                        nc.tensor.matmul(
                            acc[:], w_sb[wl][:, jj], xt[(xl, js[jj])][:, :, tsl],
                            start=(n == 0), stop=(n == nmm - 1),
                            perf_mode=DR)
                        n += 1
                    dst = qk_sb[(part, h)]
                    if ps == 0:
                        nc.scalar.copy(dst[:, tsl], acc[:])
                    else:
                        nc.vector.tensor_tensor(dst[:, tsl], dst[:, tsl],
                                                acc[:], op=ADD)
                # two v token-tiles after each q/k group (the last four
                # pass-B tiles are deferred into block-0 attention)
                for tt in (2 * gi, 2 * gi + 1):
                    if ps == 1 and tt >= 12:
                        continue
                    ssl = slice(tt * P, (tt + 1) * P)
                    vacc = psv.tile([P, 512], F32, tag="v", bufs=3)
                    n = 0
                    for xl, wl in ((1, 1), (1, 2), (2, 1)):
                        for jj, j in enumerate(js):
                            nc.tensor.matmul(
                                vacc[:], xt[(xl, j)][:, :, ssl], wv_sb[wl][:, j],
                                start=(n == 0), stop=(n == nmm - 1),
                                perf_mode=DR)
                            n += 1
                    if ps == 0:
                        nc.scalar.copy(v_sb[tt][:], vacc[:])
                    else:
                        nc.vector.tensor_tensor(v_sb[tt][:], v_sb[tt][:],
                                                vacc[:], op=ADD)
                if ps == 1:
                    rope0(part, h)
    return wv_sb


def _attention_head(nc, pools, qk_sb, v_sb, consts, jb, h, interleave,
                    head_start=None):
    """Scores/exp/mask/AV/l for one (jb, h), with score pipelining and
    outproj interleave. head_start (the previous head's deferred
    normalize chain) is emitted after this head's first two scores so
    its PE/DVE ops hide behind fresh score work."""
    pss, psc, psl, ppool, rrpool = pools
    tri_sb, onescol_sb = consts
    qT = qk_sb[("q", h)]
    qsl = slice(jb * 512, (jb + 1) * 512)
    nt = 4 * (jb + 1)
    ctx_ps = psc.tile([P, 512], F32, tag="ctx", bufs=2)
    l_ps = psl.tile([1, 512], F32, tag="l", bufs=1)

    def score(i):
        r = i - 4 * jb
        c0 = max(0, r * P)
        osl = slice(c0, 512)
        sps = pss.tile([P, 512], F32, tag="s", bufs=2)
        kT = qk_sb[("k", h)]
        nc.tensor.matmul(sps[:, osl], kT[:, i * P:(i + 1) * P],
                         qT[:, jb * 512 + c0:(jb + 1) * 512],
                         start=True, stop=True)
        pt = ppool.tile([P, 512], BF, tag="pt", bufs=5)
        nc.scalar.activation(pt[:, osl], sps[:, osl], EXP, scale=ESC)
        if r >= 0:
            dsl = slice(r * P, (r + 1) * P)
            nc.gpsimd.tensor_tensor(pt[:, dsl], pt[:, dsl], tri_sb[:], op=MULT)
        return pt, c0

    ahead = 6
    queue = [score(i) for i in range(min(ahead, nt))]
    for i in range(nt):
        pt, c0 = queue.pop(0)
        if i + ahead < nt:
            queue.append(score(i + ahead))
        if i == 0 and head_start is not None:
            head_start()
        osl = slice(c0, 512)
        nc.tensor.matmul(ctx_ps[:, osl], v_sb[i][:, h * P:(h + 1) * P],
                         pt[:, osl], start=(i == 0), stop=(i == nt - 1))
        nc.tensor.matmul(l_ps[:, osl], onescol_sb[:], pt[:, osl],
                         start=(i == 0), stop=(i == nt - 1))
        interleave()
    rinv_row = rrpool.tile([1, 512], BF, tag="rr", bufs=2)
    with nc.allow_low_precision(reason="softmax 1/l bf16"):
        nc.vector.reciprocal(rinv_row[:], l_ps[:])
    return ctx_ps, rinv_row


def _normalize_ctx(nc, pools, consts, ctx_ps, rinv_row, c1t, c2t, sl):
    """broadcast 1/l -> normalize -> split ctx into fp8 hi/lo pair slots."""
    psb, rrpool, cxnpool = pools
    onesrow_sb, = consts
    bps = rrpool.tile([P, 512], BF, tag="bb", bufs=2, name="bps_sb")
    nc.gpsimd.partition_broadcast(bps[:], rinv_row[:])
    # bps lives in SBUF, so DVE can read the PSUM ctx directly (one PSUM
    # operand per op): the fp8 hi part and the f32 normalized ctx are
    # independent products of the same operands, then lo = f32 - hi.
    ctxn = cxnpool.tile([P, 512], F32, tag="cxn", bufs=2)
    nc.vector.tensor_tensor(ctxn[:], ctx_ps[:], bps[:], op=MULT)
    nc.vector.tensor_copy(c1t[:, sl], ctxn[:])
    nc.vector.tensor_tensor(c2t[:, sl], ctxn[:], c1t[:, sl], op=SUB)


def _phase2(nc, tc, dram, qk_sb, v_sb, gtiles, vdefer):
    wo1, wo2, y = dram
    tri_sb, onescol_sb, onesrow_sb = gtiles
    with (
        tc.tile_pool(name="wop", bufs=1) as wopool,
        tc.tile_pool(name="ctx1p", bufs=2) as c1pool,
        tc.tile_pool(name="ctx2p", bufs=2) as c2pool,
        tc.tile_pool(name="cxn", bufs=2) as cxnpool,
        tc.tile_pool(name="rrow", bufs=2) as rrpool,
        tc.tile_pool(name="pp", bufs=4) as ppool,
        tc.tile_pool(name="yp", bufs=3) as ypool,
        tc.tile_pool(name="pss", bufs=2, space="PSUM") as pss,
        tc.tile_pool(name="psm", bufs=1, space="PSUM") as psm,
        tc.tile_pool(name="psc", bufs=2, space="PSUM") as psc,
        tc.tile_pool(name="psy", bufs=3, space="PSUM") as psy,
    ):
        wo_sb = {}

        def load_wo():
            for hp in range(2):
                for lv, ten in ((1, wo1), (2, wo2)):
                    t_ = wopool.tile([P, 2, C], F8, tag=f"wo{hp}_{lv}")
                    nc.sync.dma_start(t_[:], ten[hp])
                    wo_sb[(hp, lv)] = t_

        ctx1 = {}   # (jb, hp) -> [P, 2, 512] fp8
        ctx2 = {}
        yrow = {}

        def outproj_thunk(jb, sub, ob):
            # two pops per (sub, ob): 3 matmuls each, sharing one open
            # PSUM accumulation - finer interleave granularity
            box = {}

            def run_a():
                yps = psy.tile([P, 512], F32, tag="y", bufs=3, name="yps")
                box["yps"] = yps
                ssl = slice(sub * P, (sub + 1) * P)
                osl = slice(ob * 512, (ob + 1) * 512)
                for n, (ct, wl) in enumerate(((ctx1, 1), (ctx1, 2), (ctx2, 1))):
                    nc.tensor.matmul(
                        yps[:], ct[(jb, 0)][:, :, ssl],
                        wo_sb[(0, wl)][:, :, osl],
                        start=(n == 0), stop=False, perf_mode=DR)

            def run():
                tt = jb * 4 + sub
                ssl = slice(sub * P, (sub + 1) * P)
                osl = slice(ob * 512, (ob + 1) * 512)
                yps = box["yps"]
                for n, (ct, wl) in enumerate(((ctx1, 1), (ctx1, 2), (ctx2, 1))):
                    nc.tensor.matmul(
                        yps[:], ct[(jb, 1)][:, :, ssl],
                        wo_sb[(1, wl)][:, :, osl],
                        start=False, stop=(n == 2), perf_mode=DR)
                if ob == 0:
                    yrow[tt] = ypool.tile([P, T], BF, tag="ysb", bufs=3,
                                          name=f"yrow{tt}")
                y_sb = yrow[tt]
                if (sub + ob) % 2 == 0:
                    nc.vector.tensor_scalar_mul(y_sb[:, osl], yps[:], 1.0 / WS)
                else:
                    nc.scalar.mul(y_sb[:, osl], yps[:], 1.0 / WS)
                if jb == NB - 1 and sub == NB - 1:
                    nc.sync.dma_start(y[tt * P:(tt + 1) * P, osl],
                                      y_sb[:, osl])
                elif ob == NB - 1:
                    nc.sync.dma_start(y[tt * P:(tt + 1) * P, :], y_sb[:])
            return run_a, run

        xtail, wv_sb = vdefer

        def v_thunk(tt):
            def run():
                ssl = slice(tt * P - 3 * 512, tt * P - 3 * 512 + P)
                vacc = psy.tile([P, 512], F32, tag="y", bufs=3, name="vacc")
                n = 0
                for xl, wl in ((1, 1), (1, 2), (2, 1)):
                    for j in range(2, NPAIR):
                        nc.tensor.matmul(
                            vacc[:], xtail[(xl, j)][:, :, ssl], wv_sb[wl][:, j],
                            start=(n == 0), stop=(n == 17), perf_mode=DR)
                        n += 1
                nc.vector.tensor_tensor(v_sb[tt][:], v_sb[tt][:], vacc[:],
                                        op=ADD)
            return run

        pending = [v_thunk(tt) for tt in range(12, NTT)]
        ahead = {"pend": pending, "it": 0, "niter": 0,
                 "npend": 0, "done": 0}

        def interleave():
            # Bresenham spread: pop so that after it iterations,
            # floor(it * npend / niter) thunks have run
            ahead["it"] += 1
            if not ahead["pend"] or not ahead["niter"]:
                return
            want = (ahead["it"] * ahead["npend"]) // ahead["niter"]
            while ahead["done"] < want and ahead["pend"]:
                ahead["pend"].pop(0)()
                ahead["done"] += 1

        att_pools = (pss, psc, psm, ppool, rrpool)
        att_consts = (tri_sb, onescol_sb)
        nrm_pools = (None, rrpool, cxnpool)
        nrm_consts = (onesrow_sb,)

        load_wo()
        xfin = [None]

        for jb in range(NB):
            if xfin[0] is not None:
                xfin[0]()
                xfin[0] = None

            nt = 4 * (jb + 1)
            ahead["it"] = 0
            ahead["done"] = 0
            ahead["niter"] = HPC * nt
            ahead["npend"] = len(pending)

            fin = None
            for h in range(HPC):
                ctx_ps, rinv_row = _attention_head(
                    nc, att_pools, qk_sb, v_sb, att_consts, jb, h, interleave,
                    head_start=fin)
                hp, sl = h // 2, h % 2
                if sl == 0:
                    ctx1[(jb, hp)] = c1pool.tile(
                        [P, 2, 512], F8, tag=f"c1_{hp}", bufs=2,
                        name=f"c1_{jb}_{hp}")
                    ctx2[(jb, hp)] = c2pool.tile(
                        [P, 2, 512], F8, tag=f"c2_{hp}", bufs=2,
                        name=f"c2_{jb}_{hp}")

                def fin(cp=ctx_ps, rr=rinv_row, c1t=ctx1[(jb, hp)],
                        c2t=ctx2[(jb, hp)], s=sl):
                    _normalize_ctx(nc, nrm_pools, nrm_consts, cp, rr,
                                   c1t, c2t, s)
            if jb + 1 < NB:
                xfin[0] = fin
            else:
                fin()

            while pending:
                pending.pop(0)()
            for sub in range(4):
                for ob in range(4):
                    pending.extend(outproj_thunk(jb, sub, ob))
            ahead["pend"] = pending

        while pending:
            pending.pop(0)()


def _build():
    nc = bacc.Bacc("TRN2", target_bir_lowering=False, debug=False,
                   num_devices=NCORES)
    x1 = nc.dram_tensor("x1", (NPAIR, P, 2, T), F8, kind="ExternalInput").ap()
    x2 = nc.dram_tensor("x2", (NPAIR, P, 2, T), F8, kind="ExternalInput").ap()
    wq1 = nc.dram_tensor("wq1", (HPC, P, NPAIR, 2, P), F8, kind="ExternalInput").ap()
    wq2 = nc.dram_tensor("wq2", (HPC, P, NPAIR, 2, P), F8, kind="ExternalInput").ap()
    wk1 = nc.dram_tensor("wk1", (HPC, P, NPAIR, 2, P), F8, kind="ExternalInput").ap()
    wk2 = nc.dram_tensor("wk2", (HPC, P, NPAIR, 2, P), F8, kind="ExternalInput").ap()
    wv1 = nc.dram_tensor("wv1", (P, NPAIR, 2, HPC * P), F8, kind="ExternalInput").ap()
    wv2 = nc.dram_tensor("wv2", (P, NPAIR, 2, HPC * P), F8, kind="ExternalInput").ap()
    wo1 = nc.dram_tensor("wo1", (2, P, 2, C), F8, kind="ExternalInput").ap()
    wo2 = nc.dram_tensor("wo2", (2, P, 2, C), F8, kind="ExternalInput").ap()
    cosT = nc.dram_tensor("cosT", (P, T), BF, kind="ExternalInput").ap()
    sinT = nc.dram_tensor("sinT", (P, T), BF, kind="ExternalInput").ap()
    tri = nc.dram_tensor("tri", (P, P), BF, kind="ExternalInput").ap()
    onescol = nc.dram_tensor("onescol", (P, 1), BF, kind="ExternalInput").ap()
    onesrow = nc.dram_tensor("onesrow", (1, P), BF, kind="ExternalInput").ap()
    y = nc.dram_tensor("y", (T, C), BF, kind="ExternalOutput").ap()

    with tile.TileContext(nc) as tc:
        with (
            tc.tile_pool(name="gconst", bufs=1) as gpool,
            tc.tile_pool(name="qkbuf", bufs=1) as qkpool,
            tc.tile_pool(name="vbuf", bufs=1) as vpool,
        ):
            tri_sb = gpool.tile([P, P], BF, tag="tri")
            onescol_sb = gpool.tile([P, 1], BF, tag="ocol")
            onesrow_sb = gpool.tile([1, P], BF, tag="orow")
            cos_sb = gpool.tile([P, T], BF, tag="cos")
            sin_sb = gpool.tile([P, T], BF, tag="sin")

            def const_loads():
                nc.sync.dma_start(cos_sb[:], cosT)
                nc.sync.dma_start(sin_sb[:], sinT)
                nc.sync.dma_start(tri_sb[:], tri)
                nc.sync.dma_start(onescol_sb[:], onescol)
                nc.sync.dma_start(onesrow_sb[:], onesrow)

            wvpool_cm = tc.tile_pool(name="wvp", bufs=1)
            wvpool = wvpool_cm.__enter__()
            xtpool_cm = tc.tile_pool(name="xtail", bufs=1)
            xtpool = xtpool_cm.__enter__()
            xtail = {}

            def xtail_loads():
                for j in range(2, NPAIR):
                    for lv, ten in ((1, x1), (2, x2)):
                        t_ = xtpool.tile([P, 2, 512], F8, tag=f"xt{lv}_{j}",
                                         name=f"xt{lv}_{j}")
                        nc.sync.dma_start(t_[:], ten[j][:, :, 3 * 512:])
                        xtail[(lv, j)] = t_

            spool_cm = tc.tile_pool(name="st", bufs=2)
            spool = spool_cm.__enter__()
            half = P // 2

            def rope_row(part, h):
                # whole-row rope for (part, h): the half-swap is 2 big DMAs
                # instead of 8 small ones (descriptor floor dominates small
                # transfers), and the mul/mul/add chain is 3 ops per row
                src = qk_sb[(part, h)]
                tmp = spool.tile([P, T], BF, tag="rt", bufs=2, name="rtmp")
                t1 = spool.tile([P, T], BF, tag="t1", bufs=2, name="rt1")
                t2 = spool.tile([P, T], BF, tag="t2", bufs=2, name="rt2")
                nc.sync.dma_start(tmp[0:half, :], src[half:P, :])
                nc.sync.dma_start(tmp[half:P, :], src[0:half, :])
                nc.gpsimd.tensor_tensor(t1[:], src[:], cos_sb[:], op=MULT)
                nc.vector.tensor_tensor(t2[:], tmp[:], sin_sb[:], op=MULT)
                nc.vector.tensor_tensor(src[:], t1[:], t2[:], op=ADD)

            qk_sb = {}
            for h in range(HPC):
                for part in ("q", "k"):
                    qk_sb[(part, h)] = qkpool.tile(
                        [P, T], BF, tag=f"{part}{h}",
                        name=f"{part}{h}_sb")
            v_sb = [vpool.tile([P, HPC * P], BF, tag=f"vb{i}", name=f"v{i}_sb")
                    for i in range(NTT)]

            wv_sb = _phase1(nc, tc, (x1, x2, wq1, wq2, wk1, wk2, wv1, wv2),
                            qk_sb, v_sb, const_loads, rope_row, wvpool,
                            xtail_loads)
            spool_cm.__exit__(None, None, None)
            _phase2(nc, tc, (wo1, wo2, y), qk_sb, v_sb,
                    (tri_sb, onescol_sb, onesrow_sb), (xtail, wv_sb))
            xtpool_cm.__exit__(None, None, None)
            wvpool_cm.__exit__(None, None, None)

    nc.compile()
    return nc


def _build_kernel():
    if "k" not in _CACHE:
        _CACHE["k"] = _build()
    return _CACHE["k"]


E4NP = ml_dtypes.float8_e4m3
BFNP = ml_dtypes.bfloat16


def _split8(a):
    a1 = np.asarray(a, np.float32).astype(E4NP)
    a2 = (np.asarray(a, np.float32) - a1.astype(np.float32)).astype(E4NP)
    return a1, a2


def prepare_in_maps(x, W_qkv, W_o, cos, sin):
    tri01 = (np.arange(P)[:, None] <= np.arange(P)[None, :]).astype(BFNP)
    onescol = np.full((P, 1), WS, dtype=np.float32).astype(BFNP)
    onesrow = np.ones((1, P), dtype=np.float32).astype(BFNP)
    cosT = np.ascontiguousarray(cos.T).astype(BFNP)
    sgn = np.where(np.arange(P) < P // 2, -1.0, 1.0).astype(np.float32)
    sinT = np.ascontiguousarray(sin.T * sgn[:, None]).astype(BFNP)

    # per-batch x fp8 pair chunks [NPAIR, P, 2, T]
    xq = {}
    for b in range(2):
        xT = np.ascontiguousarray(x[b].T)
        xs = _split8(xT)
        xq[b] = tuple(
            np.ascontiguousarray(
                a.reshape(NPAIR, 2, P, T).transpose(0, 2, 1, 3))
            for a in xs)

    def wqk_layout(a):   # [C, 512] -> (HPC, P, NPAIR, 2, P)
        return np.ascontiguousarray(
            a.reshape(NPAIR, 2, P, HPC, P).transpose(3, 2, 0, 1, 4))

    def wv_layout(a):    # [C, 512] -> (P, NPAIR, 2, HPC*P)
        return np.ascontiguousarray(
            a.reshape(NPAIR, 2, P, HPC * P).transpose(2, 0, 1, 3))

    in_maps = []
    for core in range(NCORES):
        b = core // 4
        hg0 = (core % 4) * HPC
        rows = slice(hg0 * P, (hg0 + HPC) * P)
        wq_r = WS * W_qkv[0 * C:1 * C][rows].T   # [C, 512]
        wk_r = WS * W_qkv[1 * C:2 * C][rows].T
        wv_r = WS * W_qkv[2 * C:3 * C][rows].T
        wq1, wq2 = (wqk_layout(a) for a in _split8(wq_r))
        wk1, wk2 = (wqk_layout(a) for a in _split8(wk_r))
        wv1, wv2 = (wv_layout(a) for a in _split8(wv_r))
        wo_r = WS * W_o[:, rows].T               # [512, C]
        wo1, wo2 = (
            np.ascontiguousarray(a.reshape(2, 2, P, C).transpose(0, 2, 1, 3))
            for a in _split8(wo_r))
        in_maps.append({
            "x1": xq[b][0], "x2": xq[b][1],
            "wq1": wq1, "wq2": wq2, "wk1": wk1, "wk2": wk2,
            "wv1": wv1, "wv2": wv2, "wo1": wo1, "wo2": wo2,
            "cosT": cosT, "sinT": sinT, "tri": tri01,
            "onescol": onescol, "onesrow": onesrow,
        })
    return in_maps


def gather(results, b_o):
    y = np.zeros((2, T, C), dtype=np.float32)
    for core in range(NCORES):
        y[core // 4] += np.asarray(results[core]["y"], dtype=np.float32)
    y += np.asarray(b_o, dtype=np.float32)[None, None, :]
    return y


def kernel(x, W_qkv, W_o, b_o, cos, sin):
    x = np.asarray(x, dtype=np.float32)
    W_qkv = np.asarray(W_qkv, dtype=np.float32)
    W_o = np.asarray(W_o, dtype=np.float32)
    cos = np.asarray(cos, dtype=np.float32)
    sin = np.asarray(sin, dtype=np.float32)
    nc = _build_kernel()
    in_maps = prepare_in_maps(x, W_qkv, W_o, cos, sin)
    res = run_bass_kernel_spmd(nc, in_maps, core_ids=list(range(NCORES)))
    return gather(res.results, b_o)
